# revision 39
# baseline (speedup 1.0000x reference)
"""Trainium2 Bass kernel for imagen-style self-attention with pos_bias.

Reference computation (fp32 jax):
    xn   = LN(x) * g_norm                      # gamma-only layernorm
    qkv  = xn @ w_qkv ; q,k,v per head (h=8, d=64) ; q *= d**-0.5
    sim  = q @ k^T + pos_bias[h]               # [b, h, n, n]
    attn = softmax(sim, -1)
    out  = LN((attn @ v) @ w_out) * g_out

Distribution: 8 cores, one head per core (tensor parallel over heads).
Each core computes LN+QKV projection for its head over the full batch,
full attention for its head, then AllToAlls (split into n/512 chunks so
comm overlaps compute) re-shard by sequence rows; each core runs the
output projection for a 1/8 row shard; the final LN runs batched at the
end (keeps the scalar engine's Exp activation table resident during the
whole attention phase).

Row shard mapping: block (b, ii) = rows (b, i in [512*ii, 512*ii+512))
is split into two 256-row halves h; half (b, ii, h) goes to core 2b+h.
So core c owns rows (b=c//2, i in [512*ii + 256*(c%2), +256)) for all
ii, in ii order.

Softmax denominators travel with the AllToAll payload (row DH of each
[DH+1, 256] chunk); the attention output is scaled by 2^-6 pre-cast so
unnormalized f16 values cannot overflow, and normalization happens on
the receiving side (scale cancels in o/sums).
"""

import numpy as np

import concourse.bass as bass
import concourse.bacc as bacc
import concourse.mybir as mybir
import concourse.tile as tile
from concourse.bass_utils import run_bass_kernel_spmd
from concourse.masks import make_identity

B = 4
N = 2048
D = 512
HEADS = 8
DH = 64
SCALE = DH**-0.5
EPS = 1e-5
NCORES = 8
OSCALE = 2.0**-6

F16 = mybir.dt.float16
F32 = mybir.dt.float32
AF = mybir.ActivationFunctionType
ALU = mybir.AluOpType


def _phase1(nc, tc, x_d, w_sb, eps_t, ident, qT_sb, kT_sb, v_sb, n, b):
    """LN + transpose + QKV projection over all rows."""
    rows = b * n
    n_spans = rows // 512
    with (
        tc.tile_pool(name="p1", bufs=3) as p1,
        tc.tile_pool(name="p1xT", bufs=2) as p1xT,
        tc.tile_pool(name="ps_t", bufs=2, space="PSUM") as ps_t,
        tc.tile_pool(name="ps_p", bufs=2, space="PSUM") as ps_p,
    ):
        for sp in range(n_spans):
            xnT = p1xT.tile([128, 4, 512], F16, tag="xnT")
            mvs = p1.tile([128, 4, 2], F32, tag="mvs")
            rstds = p1.tile([128, 4], F32, tag="rstds")
            x_ts = []
            for t in range(4):
                rt = sp * 4 + t
                x_t = p1.tile([128, D], F16, tag=f"x{t}", name=f"x{t}")
                x_ts.append(x_t)
                nc.sync.dma_start(out=x_t, in_=x_d[rt * 128 : (rt + 1) * 128, :])
                stats = p1.tile([128, 6], F32, tag="stats")
                nc.vector.bn_stats(out=stats, in_=x_t)
                nc.vector.bn_aggr(out=mvs[:, t, :], in_=stats)
            # batched rstd for the whole span: 1/sqrt(var + eps)
            nc.scalar.activation(
                out=rstds, in_=mvs[:, :, 1], func=AF.Sqrt, bias=eps_t
            )
            nc.vector.reciprocal(out=rstds, in_=rstds)
            for t in range(4):
                xn_t = p1.tile([128, D], F16, tag="xn")
                nc.vector.tensor_scalar(
                    out=xn_t,
                    in0=x_ts[t],
                    scalar1=mvs[:, t, 0:1],
                    scalar2=rstds[:, t : t + 1],
                    op0=ALU.subtract,
                    op1=ALU.mult,
                )
                # transpose 4 chunks into one psum bank, single evac cast
                ps = ps_t.tile([128, 512], F32, tag="tp")
                for c in range(4):
                    nc.tensor.matmul(
                        ps[:, c * 128 : (c + 1) * 128],
                        lhsT=xn_t[:, c * 128 : (c + 1) * 128],
                        rhs=ident,
                        start=(c == 0),
                        stop=(c == 3),
                    )
                nc.scalar.copy(
                    out=xnT[:, :, t * 128 : (t + 1) * 128],
                    in_=ps.rearrange("p (c w) -> p c w", c=4),
                )

            bi = (sp * 512) // n
            cols = slice((sp * 512) % n, (sp * 512) % n + 512)
            ps_q = ps_p.tile([64, 512], F32, tag="q")
            ps_k = ps_p.tile([64, 512], F32, tag="k")
            ps_v = ps_p.tile([64, 512], F32, tag="v")
            for c in range(4):
                st, fin = (c == 0), (c == 3)
                nc.tensor.matmul(
                    ps_q, lhsT=w_sb[:, c, 0:64], rhs=xnT[:, c, :], start=st, stop=fin
                )
                nc.tensor.matmul(
                    ps_k, lhsT=w_sb[:, c, 64:128], rhs=xnT[:, c, :], start=st, stop=fin
                )
                nc.tensor.matmul(
                    ps_v, lhsT=w_sb[:, c, 128:192], rhs=xnT[:, c, :], start=st, stop=fin
                )
            nc.vector.tensor_copy(out=qT_sb[:, bi, cols], in_=ps_q)
            nc.scalar.copy(out=kT_sb[:, bi, cols], in_=ps_k)
            vT_t = p1.tile([64, 512], F16, tag="vT")
            nc.scalar.copy(out=vT_t, in_=ps_v)
            # transpose vT [64, 512] into v natural [512, 64]: 4 transposes
            # into one psum bank, single strided evac
            jc0 = ((sp * 512) % n) // 128
            ps2 = ps_t.tile([128, 4, 64], F32, tag="tp", name="psv")
            for t in range(4):
                nc.tensor.matmul(
                    ps2[:, t, :],
                    lhsT=vT_t[:, t * 128 : (t + 1) * 128],
                    rhs=ident[0:64, 0:64],
                    start=(t == 0),
                    stop=(t == 3),
                )
            nc.scalar.copy(out=v_sb[:, bi, jc0 : jc0 + 4, 0:DH], in_=ps2)


def _load_pos(nc, p2c, post_d, n, ii, js=None):
    """Start DMA of the pos_bias column block for one i-span."""
    n_jc = n // 128
    pos_c = p2c.tile([128, n_jc, 512], F16, tag="posc")
    icols = slice(ii * 512, (ii + 1) * 512)
    for j in js if js is not None else range(n_jc):
        nc.sync.dma_start(
            out=pos_c[:, j, :], in_=post_d[j * 128 : (j + 1) * 128, icols]
        )
    return pos_c


def _load_pos_rest(nc, pos_c, post_d, n, ii, js):
    icols = slice(ii * 512, (ii + 1) * 512)
    for j in js:
        nc.sync.dma_start(
            out=pos_c[:, j, :], in_=post_d[j * 128 : (j + 1) * 128, icols]
        )


def _phase2(nc, pools, ident, qT_sb, kT_sb, v_sb, a2a_ins, n, b, ii, pos_c):
    """Attention for one i-span: S^T = pos^T + kT.T@qT ; exp ; O^T.

    The pos broadcast rides the PE (identity matmul into the S psum)
    deliberately: it keeps the PE the saturated rate-limiting engine,
    which holds the HAM clock gate open — scalar-bound variants let the
    PE idle per-pair and it gets throttled to 1.2 GHz, costing far more
    than the broadcast matmuls. j-chunk pairs share one [128, 2, 512] S
    psum (2 banks) so exp runs 1024 wide; O is left unnormalized
    (scaled by OSCALE) and the softmax sums row travels with the a2a
    payload."""
    n_jc = n // 128
    p2, p2o, ps_s, ps_o = pools
    icols = slice(ii * 512, (ii + 1) * 512)
    for bi in range(b):
        ps_O = ps_o.tile([DH + 1, 512], F32, tag="O")
        p_ts = {}

        def _o_mm(g):
            p_t = p_ts.pop(g)
            for h in range(2):
                j = 2 * g + h
                nc.tensor.matmul(
                    ps_O,
                    lhsT=v_sb[:, bi, j, :],
                    rhs=p_t[:, h, :],
                    start=(j == 0),
                    stop=(j == n_jc - 1),
                )

        for g in range(n_jc // 2):
            ps_S = ps_s.tile([128, 2, 512], F32, tag="S")
            p_t = p2.tile([128, 2, 512], F16, tag="P")
            p_ts[g] = p_t
            for h in range(2):
                nc.tensor.matmul(
                    ps_S[:, h, :],
                    lhsT=ident,
                    rhs=pos_c[:, 2 * g + h, :],
                    start=True,
                    stop=False,
                )
            for h in range(2):
                j = 2 * g + h
                nc.tensor.matmul(
                    ps_S[:, h, :],
                    lhsT=kT_sb[:, bi, j * 128 : (j + 1) * 128],
                    rhs=qT_sb[:, bi, icols],
                    start=False,
                    stop=True,
                )
            nc.scalar.activation(out=p_t, in_=ps_S, func=AF.Exp)
            if g >= 1:
                _o_mm(g - 1)
        _o_mm(n_jc // 2 - 1)
        o_t = p2o.tile([DH + 1, 512], F16, tag="onorm")
        nc.vector.tensor_scalar_mul(o_t, ps_O, OSCALE)
        for h in range(2):
            nc.sync.dma_start(
                out=a2a_ins[ii][2 * bi + h, :, :],
                in_=o_t[:, h * 256 : (h + 1) * 256],
            )


def _phase4_proj(nc, pools, src, wout_sb, sel_sb, y_sb, ci):
    """Out projection for one 256-row a2a chunk (normalize by sums row)."""
    p4, p4h, ps_r, ps_y = pools
    hT = p4h.tile([128, 4, 256], F16, tag="hT")
    for c in range(4):
        for two in range(2):
            nc.sync.dma_start(
                out=hT[two * DH : (two + 1) * DH, c, :],
                in_=src[2 * c + two, 0:DH, :],
            )
    sums = p4.tile([8, 256], F16, tag="sums")
    nc.sync.dma_start(out=sums, in_=src[:, DH, :])
    rec = p4.tile([8, 256], F16, tag="rec")
    with nc.allow_low_precision(reason="f16 softmax denom recip, tol 2e-2"):
        nc.vector.reciprocal(out=rec, in_=sums)
    hTn = p4h.tile([128, 4, 256], F16, tag="hTn")
    for cc in range(2):
        ps_rb = ps_r.tile([128, 2, 256], F32, tag="rb")
        for k in range(2):
            c = 2 * cc + k
            # broadcast rec rows into the partition layout of hT via PE:
            # sel_sb[:, c, :][s, p] = (s == 2c + p//64)
            nc.tensor.matmul(
                ps_rb[:, k, :], lhsT=sel_sb[:, c, :], rhs=rec, start=True, stop=True
            )
        for k in range(2):
            c = 2 * cc + k
            nc.vector.tensor_tensor(
                out=hTn[:, c, :], in0=hT[:, c, :], in1=ps_rb[:, k, :], op=ALU.mult
            )
    for it in range(2):
        ps = ps_y.tile([128, D], F32, tag="y")
        for c in range(4):
            nc.tensor.matmul(
                ps,
                lhsT=hTn[:, c, it * 128 : (it + 1) * 128],
                rhs=wout_sb[:, c, :],
                start=(c == 0),
                stop=(c == 3),
            )
        nc.vector.tensor_copy(out=y_sb[:, 2 * ci + it, :], in_=ps)


def _phase5_ln(nc, p5, y_sb, g_bc, eps_t, out_d, chunks):
    """Final LN over the given row chunks, batched (Sqrt table stays
    clear of the phase2 Exp stream)."""
    nch = len(chunks)
    mvs = p5.tile([128, 8, 2], F32, tag="mv5")
    for i, t in enumerate(chunks):
        stats = p5.tile([128, 6], F32, tag="st5")
        nc.vector.bn_stats(out=stats, in_=y_sb[:, t, :])
        nc.vector.bn_aggr(out=mvs[:, i, :], in_=stats)
    rstds = p5.tile([128, 8], F32, tag="rstd5")
    nc.scalar.activation(
        out=rstds[:, 0:nch], in_=mvs[:, 0:nch, 1], func=AF.Sqrt, bias=eps_t
    )
    nc.vector.reciprocal(out=rstds[:, 0:nch], in_=rstds[:, 0:nch])
    for i, t in enumerate(chunks):
        y_t = p5.tile([128, D], F32, tag="y5")
        nc.vector.tensor_scalar(
            out=y_t,
            in0=y_sb[:, t, :],
            scalar1=mvs[:, i, 0:1],
            scalar2=rstds[:, i : i + 1],
            op0=ALU.subtract,
            op1=ALU.mult,
        )
        nc.vector.tensor_tensor(out=y_t, in0=y_t, in1=g_bc, op=ALU.mult)
        nc.sync.dma_start(out=out_d[t * 128 : (t + 1) * 128, :], in_=y_t)


def build_attention_bass(n: int = N, b: int = B) -> bass.Bass:
    """Build the SPMD per-core Bass program (identical on all cores)."""
    rows = b * n
    assert rows % (NCORES * 128) == 0 and n % 512 == 0 and b == 4
    rows_pc = rows // NCORES
    n_ii = n // 512
    n_jc = n // 128

    nc = bacc.Bacc(num_devices=NCORES)

    x_d = nc.declare_dram_parameter("x", [rows, D], F16, isOutput=False)
    w_d = nc.declare_dram_parameter("w", [4, 128, 3 * DH], F16, isOutput=False)
    post_d = nc.declare_dram_parameter("post", [n, n], F16, isOutput=False)
    wout_d = nc.declare_dram_parameter("wout", [4, 128, D], F16, isOutput=False)
    g_d = nc.declare_dram_parameter("g", [1, D], F32, isOutput=False)
    sel_d = nc.declare_dram_parameter("sel", [4, 8, 128], F16, isOutput=False)
    out_d = nc.declare_dram_parameter("out", [rows_pc, D], F32, isOutput=True)

    a2a_ins = [
        nc.dram_tensor(f"a2a_in{ii}", [NCORES, DH + 1, 256], F16)
        for ii in range(n_ii)
    ]
    a2a_outs = [
        nc.dram_tensor(f"a2a_out{ii}", [NCORES, DH + 1, 256], F16)
        for ii in range(n_ii)
    ]
    warm_in = nc.dram_tensor("warm_in", [NCORES, 16], F16)
    warm_out = nc.dram_tensor("warm_out", [NCORES, 16], F16)

    groups = [list(range(NCORES))]

    with tile.TileContext(nc) as tc:
        with (
            tc.tile_pool(name="singles", bufs=1) as singles,
            tc.tile_pool(name="persist", bufs=1) as persist,
            tc.tile_pool(name="p2c", bufs=2) as p2c,
        ):
            # warm up the collective channels while phase1 computes
            nc.gpsimd.collective_compute(
                "AllToAll",
                ALU.bypass,
                replica_groups=groups,
                ins=[warm_in[:]],
                outs=[warm_out[:]],
            )
            ident = singles.tile([128, 128], F16)
            make_identity(nc, ident)
            eps_t = singles.tile([128, 1], F32)
            nc.vector.memset(eps_t, EPS)
            w_sb = singles.tile([128, 4, 3 * DH], F16)
            nc.sync.dma_start(out=w_sb, in_=w_d.rearrange("c p m -> p c m"))

            qT_sb = persist.tile([64, b, n], F16, name="qT")
            kT_sb = persist.tile([64, b, n], F16, name="kT")
            v_sb = persist.tile([128, b, n_jc, DH + 1], F16, name="v")
            nc.vector.memset(v_sb[:, :, :, DH : DH + 1], 1.0)

            # only the first j-chunks of pos[0] load before phase1 (keeps
            # the DMA queues clear for phase1's x tiles); the rest queue
            # right after phase1's emission, still well ahead of use
            pos_cur = _load_pos(nc, p2c, post_d, n, 0, js=range(0, 4))

            with (
                tc.tile_pool(name="p2", bufs=3) as p2,
                tc.tile_pool(name="p2o", bufs=2) as p2o,
                tc.tile_pool(name="p4", bufs=2) as p4,
                tc.tile_pool(name="p4s", bufs=1) as p4s,
                tc.tile_pool(name="p4h", bufs=2) as p4h,
                tc.tile_pool(name="p5", bufs=2) as p5,
            ):
                wout_sb = p4s.tile([128, 4, D], F16)
                nc.sync.dma_start(
                    out=wout_sb, in_=wout_d.rearrange("c p m -> p c m")
                )
                g_bc = p4s.tile([128, D], F32)
                nc.sync.dma_start(
                    out=g_bc, in_=g_d[0, :].partition_broadcast(128)
                )
                sel_sb = p4s.tile([8, 4, 128], F16)
                nc.sync.dma_start(
                    out=sel_sb, in_=sel_d.rearrange("c s p -> s c p")
                )
                y_sb = p4s.tile([128, 2 * n_ii, D], F32)
                P4LAG = 1
                _phase1(nc, tc, x_d, w_sb, eps_t, ident, qT_sb, kT_sb, v_sb,
                        n, b)
                _load_pos_rest(nc, pos_cur, post_d, n, 0, range(4, 16))
                with (
                    tc.tile_pool(name="ps_s", bufs=2, space="PSUM") as ps_s,
                    tc.tile_pool(name="ps_o", bufs=2, space="PSUM") as ps_o,
                    tc.tile_pool(name="ps_r", bufs=1, space="PSUM") as ps_r,
                    tc.tile_pool(name="ps_y", bufs=1, space="PSUM") as ps_y,
                ):
                    p2pools = (p2, p2o, ps_s, ps_o)
                    p4pools = (p4, p4h, ps_r, ps_y)
                    pos_tiles = {0: pos_cur, 1: _load_pos(
                        nc, p2c, post_d, n, 1)}
                    for ii in range(n_ii):
                        _phase2(
                            nc, p2pools, ident, qT_sb, kT_sb, v_sb, a2a_ins,
                            n, b, ii, pos_tiles.pop(ii),
                        )
                        nc.gpsimd.collective_compute(
                            "AllToAll",
                            ALU.bypass,
                            replica_groups=groups,
                            ins=[a2a_ins[ii][:]],
                            outs=[a2a_outs[ii][:]],
                        )
                        if ii >= P4LAG:
                            _phase4_proj(
                                nc, p4pools, a2a_outs[ii - P4LAG], wout_sb,
                                sel_sb, y_sb, ii - P4LAG,
                            )
                        if ii + 2 < n_ii:
                            # prefetch the after-next pos block here, after
                            # the collective trigger, so its DMA doesn't
                            # contend with the in-flight a2a transfer
                            pos_tiles[ii + 2] = _load_pos(
                                nc, p2c, post_d, n, ii + 2
                            )
                    # final LN for all but the last a2a chunk hides inside
                    # the last collective's transfer window
                    _phase5_ln(
                        nc, p5, y_sb, g_bc, eps_t, out_d,
                        list(range(2 * n_ii - 2)),
                    )
                    for ii in range(max(0, n_ii - P4LAG), n_ii):
                        _phase4_proj(
                            nc, p4pools, a2a_outs[ii], wout_sb, sel_sb,
                            y_sb, ii,
                        )
                    _phase5_ln(
                        nc, p5, y_sb, g_bc, eps_t, out_d,
                        [2 * n_ii - 2, 2 * n_ii - 1],
                    )

    nc.finalize()
    return nc


def make_in_maps(x, pos_bias, w_qkv, w_out, g_norm, g_out, n=N, b=B):
    """Host-side shard/layout prep: per-core input maps (no math beyond
    folding the LN gamma / attention scale diagonals into the weights)."""
    rows = b * n
    x16 = np.ascontiguousarray(x.reshape(rows, D)).astype(np.float16)
    w_eff = w_qkv * g_norm[:, None].astype(np.float32)
    wout16 = np.ascontiguousarray(w_out.reshape(4, 128, D)).astype(np.float16)
    g_row = np.ascontiguousarray(g_out.reshape(1, D)).astype(np.float32)
    sel = np.zeros((4, 8, 128), dtype=np.float16)
    for c in range(4):
        sel[c, 2 * c, 0:64] = 1.0
        sel[c, 2 * c + 1, 64:128] = 1.0
    hidden = HEADS * DH
    in_maps = []
    for h in range(NCORES):
        wq = w_eff[:, h * DH : (h + 1) * DH] * SCALE
        wk = w_eff[:, hidden + h * DH : hidden + (h + 1) * DH]
        wv = w_eff[:, 2 * hidden + h * DH : 2 * hidden + (h + 1) * DH]
        w_h = np.concatenate([wq, wk, wv], axis=1).reshape(4, 128, 3 * DH)
        posT = np.ascontiguousarray(pos_bias[h].T).astype(np.float16)
        in_maps.append(
            {
                "x": x16,
                "w": np.ascontiguousarray(w_h).astype(np.float16),
                "post": posT,
                "wout": wout16,
                "g": g_row,
                "sel": sel,
            }
        )
    return in_maps


def assemble_output(results, n=N, b=B):
    """Scatter per-core row shards back to the full [b, n, D] output."""
    out = np.empty((b, n, D), dtype=np.float32)
    n_ii = n // 512
    for c in range(NCORES):
        oc = results[c]["out"]
        bi = c // 2
        for ii in range(n_ii):
            i0 = 512 * ii + 256 * (c % 2)
            out[bi, i0 : i0 + 256, :] = oc[ii * 256 : (ii + 1) * 256, :]
    return out


_NC_CACHE: dict = {}


def _get_nc(n=N, b=B):
    key = (n, b)
    if key not in _NC_CACHE:
        _NC_CACHE[key] = build_attention_bass(n, b)
    return _NC_CACHE[key]


def kernel(x, pos_bias, w_qkv, w_out, g_norm, g_out, _trace=False):
    x = np.asarray(x, dtype=np.float32)
    pos_bias = np.asarray(pos_bias, dtype=np.float32)
    w_qkv = np.asarray(w_qkv, dtype=np.float32)
    w_out = np.asarray(w_out, dtype=np.float32)
    g_norm = np.asarray(g_norm, dtype=np.float32)
    g_out = np.asarray(g_out, dtype=np.float32)
    b, n, _ = x.shape

    nc = _get_nc(n, b)
    in_maps = make_in_maps(x, pos_bias, w_qkv, w_out, g_norm, g_out, n, b)
    res = run_bass_kernel_spmd(
        nc, in_maps, core_ids=list(range(NCORES)), trace=_trace
    )
    if _trace:
        kernel.last_results = res
    return assemble_output(res.results, n, b)


# revision 40
# speedup vs baseline: 1.0324x; 1.0324x over previous
"""Trainium2 Bass kernel for imagen-style self-attention with pos_bias.

Reference computation (fp32 jax):
    xn   = LN(x) * g_norm                      # gamma-only layernorm
    qkv  = xn @ w_qkv ; q,k,v per head (h=8, d=64) ; q *= d**-0.5
    sim  = q @ k^T + pos_bias[h]               # [b, h, n, n]
    attn = softmax(sim, -1)
    out  = LN((attn @ v) @ w_out) * g_out

Distribution: 8 cores, one head per core (tensor parallel over heads).
Each core computes LN+QKV projection for its head over the full batch,
full attention for its head, then AllToAlls (split into n/512 chunks so
comm overlaps compute) re-shard by sequence rows; each core runs the
output projection for a 1/8 row shard; the final LN runs batched at the
end (keeps the scalar engine's Exp activation table resident during the
whole attention phase).

Row shard mapping: block (b, ii) = rows (b, i in [512*ii, 512*ii+512))
is split into two 256-row halves h; half (b, ii, h) goes to core 2b+h.
So core c owns rows (b=c//2, i in [512*ii + 256*(c%2), +256)) for all
ii, in ii order.

Softmax denominators travel with the AllToAll payload (row DH of each
[DH+1, 256] chunk); the attention output is scaled by 2^-6 pre-cast so
unnormalized f16 values cannot overflow, and normalization happens on
the receiving side (scale cancels in o/sums).
"""

import numpy as np

import concourse.bass as bass
import concourse.bacc as bacc
import concourse.mybir as mybir
import concourse.tile as tile
from concourse.bass_utils import run_bass_kernel_spmd
from concourse.masks import make_identity

B = 4
N = 2048
D = 512
HEADS = 8
DH = 64
SCALE = DH**-0.5
EPS = 1e-5
NCORES = 8
OSCALE = 2.0**-6

F16 = mybir.dt.float16
F32 = mybir.dt.float32
AF = mybir.ActivationFunctionType
ALU = mybir.AluOpType


def _phase1(nc, tc, x_d, w_sb, eps_t, ident, qT_sb, kT_sb, v_sb, n, b):
    """LN + transpose + QKV projection over all rows."""
    rows = b * n
    n_spans = rows // 512
    with (
        tc.tile_pool(name="p1", bufs=3) as p1,
        tc.tile_pool(name="p1xT", bufs=2) as p1xT,
        tc.tile_pool(name="ps_t", bufs=2, space="PSUM") as ps_t,
        tc.tile_pool(name="ps_p", bufs=2, space="PSUM") as ps_p,
    ):
        xv = x_d.rearrange("(q p) d -> p q d", p=128)
        for sp in range(n_spans):
            xnT = p1xT.tile([128, 4, 512], F16, tag="xnT")
            mvs = p1.tile([128, 4, 2], F32, tag="mvs")
            rstds = p1.tile([128, 4], F32, tag="rstds")
            x_sp = p1.tile([128, 4, D], F16, tag="xsp")
            nc.sync.dma_start(
                out=x_sp, in_=xv[:, sp * 4 : sp * 4 + 4, :]
            )
            for t in range(4):
                stats = p1.tile([128, 6], F32, tag="stats")
                nc.vector.bn_stats(out=stats, in_=x_sp[:, t, :])
                nc.vector.bn_aggr(out=mvs[:, t, :], in_=stats)
            # batched rstd for the whole span: 1/sqrt(var + eps)
            nc.scalar.activation(
                out=rstds, in_=mvs[:, :, 1], func=AF.Sqrt, bias=eps_t
            )
            nc.vector.reciprocal(out=rstds, in_=rstds)
            for t in range(4):
                xn_t = p1.tile([128, D], F16, tag="xn")
                nc.vector.tensor_scalar(
                    out=xn_t,
                    in0=x_sp[:, t, :],
                    scalar1=mvs[:, t, 0:1],
                    scalar2=rstds[:, t : t + 1],
                    op0=ALU.subtract,
                    op1=ALU.mult,
                )
                # transpose 4 chunks into one psum bank, single evac cast
                ps = ps_t.tile([128, 512], F32, tag="tp")
                for c in range(4):
                    nc.tensor.matmul(
                        ps[:, c * 128 : (c + 1) * 128],
                        lhsT=xn_t[:, c * 128 : (c + 1) * 128],
                        rhs=ident,
                        start=(c == 0),
                        stop=(c == 3),
                    )
                nc.scalar.copy(
                    out=xnT[:, :, t * 128 : (t + 1) * 128],
                    in_=ps.rearrange("p (c w) -> p c w", c=4),
                )

            bi = (sp * 512) // n
            cols = slice((sp * 512) % n, (sp * 512) % n + 512)
            ps_q = ps_p.tile([64, 512], F32, tag="q")
            ps_k = ps_p.tile([64, 512], F32, tag="k")
            ps_v = ps_p.tile([64, 512], F32, tag="v")
            for c in range(4):
                st, fin = (c == 0), (c == 3)
                nc.tensor.matmul(
                    ps_q, lhsT=w_sb[:, c, 0:64], rhs=xnT[:, c, :], start=st, stop=fin
                )
                nc.tensor.matmul(
                    ps_k, lhsT=w_sb[:, c, 64:128], rhs=xnT[:, c, :], start=st, stop=fin
                )
                nc.tensor.matmul(
                    ps_v, lhsT=w_sb[:, c, 128:192], rhs=xnT[:, c, :], start=st, stop=fin
                )
            nc.vector.tensor_copy(out=qT_sb[:, bi, cols], in_=ps_q)
            nc.scalar.copy(out=kT_sb[:, bi, cols], in_=ps_k)
            vT_t = p1.tile([64, 512], F16, tag="vT")
            nc.scalar.copy(out=vT_t, in_=ps_v)
            # transpose vT [64, 512] into v natural [512, 64]: 4 transposes
            # into one psum bank, single strided evac
            jc0 = ((sp * 512) % n) // 128
            ps2 = ps_t.tile([128, 4, 64], F32, tag="tp", name="psv")
            for t in range(4):
                nc.tensor.matmul(
                    ps2[:, t, :],
                    lhsT=vT_t[:, t * 128 : (t + 1) * 128],
                    rhs=ident[0:64, 0:64],
                    start=(t == 0),
                    stop=(t == 3),
                )
            nc.scalar.copy(out=v_sb[:, bi, jc0 : jc0 + 4, 0:DH], in_=ps2)


def _load_pos(nc, p2c, post_d, n, ii, js=None):
    """Start DMA of the pos_bias column block for one i-span (single
    strided transfer — dma_start dispatch on the sync engine costs
    ~640ns each, so batching matters)."""
    n_jc = n // 128
    pos_c = p2c.tile([128, n_jc, 512], F16, tag="posc")
    icols = slice(ii * 512, (ii + 1) * 512)
    srcv = post_d.rearrange("(j p) m -> p j m", p=128)
    if js is None:
        js = range(n_jc)
    nc.sync.dma_start(
        out=pos_c[:, js.start : js.stop, :],
        in_=srcv[:, js.start : js.stop, icols],
    )
    return pos_c


def _load_pos_rest(nc, pos_c, post_d, n, ii, js):
    icols = slice(ii * 512, (ii + 1) * 512)
    srcv = post_d.rearrange("(j p) m -> p j m", p=128)
    nc.sync.dma_start(
        out=pos_c[:, js.start : js.stop, :],
        in_=srcv[:, js.start : js.stop, icols],
    )


def _phase2(nc, pools, ident, qT_sb, kT_sb, v_sb, a2a_ins, n, b, ii, pos_c):
    """Attention for one i-span: S^T = pos^T + kT.T@qT ; exp ; O^T.

    The pos broadcast rides the PE (identity matmul into the S psum)
    deliberately: it keeps the PE the saturated rate-limiting engine,
    which holds the HAM clock gate open — scalar-bound variants let the
    PE idle per-pair and it gets throttled to 1.2 GHz, costing far more
    than the broadcast matmuls. j-chunk pairs share one [128, 2, 512] S
    psum (2 banks) so exp runs 1024 wide; O is left unnormalized
    (scaled by OSCALE) and the softmax sums row travels with the a2a
    payload."""
    n_jc = n // 128
    p2, p2o, ps_s, ps_o = pools
    icols = slice(ii * 512, (ii + 1) * 512)
    for bi in range(b):
        ps_O = ps_o.tile([DH + 1, 512], F32, tag="O")
        p_ts = {}

        def _o_mm(g):
            p_t = p_ts.pop(g)
            for h in range(2):
                j = 2 * g + h
                nc.tensor.matmul(
                    ps_O,
                    lhsT=v_sb[:, bi, j, :],
                    rhs=p_t[:, h, :],
                    start=(j == 0),
                    stop=(j == n_jc - 1),
                )

        for g in range(n_jc // 2):
            ps_S = ps_s.tile([128, 2, 512], F32, tag="S")
            p_t = p2.tile([128, 2, 512], F16, tag="P")
            p_ts[g] = p_t
            for h in range(2):
                nc.tensor.matmul(
                    ps_S[:, h, :],
                    lhsT=ident,
                    rhs=pos_c[:, 2 * g + h, :],
                    start=True,
                    stop=False,
                )
            for h in range(2):
                j = 2 * g + h
                nc.tensor.matmul(
                    ps_S[:, h, :],
                    lhsT=kT_sb[:, bi, j * 128 : (j + 1) * 128],
                    rhs=qT_sb[:, bi, icols],
                    start=False,
                    stop=True,
                )
            nc.scalar.activation(out=p_t, in_=ps_S, func=AF.Exp)
            if g >= 1:
                _o_mm(g - 1)
        _o_mm(n_jc // 2 - 1)
        o_t = p2o.tile([DH + 1, 512], F16, tag="onorm")
        nc.vector.tensor_scalar_mul(o_t, ps_O, OSCALE)
        nc.sync.dma_start(
            out=a2a_ins[ii][2 * bi : 2 * bi + 2, :, :],
            in_=o_t.rearrange("r (h c) -> h r c", h=2),
        )


def _phase4_proj(nc, pools, src, wout_sb, sel_sb, y_sb, ci):
    """Out projection for one 256-row a2a chunk (normalize by sums row)."""
    p4, p4h, ps_r, ps_y = pools
    hT = p4h.tile([128, 4, 256], F16, tag="hT")
    for c in range(4):
        for two in range(2):
            nc.sync.dma_start(
                out=hT[two * DH : (two + 1) * DH, c, :],
                in_=src[2 * c + two, 0:DH, :],
            )
    sums = p4.tile([8, 256], F16, tag="sums")
    nc.sync.dma_start(out=sums, in_=src[:, DH, :])
    rec = p4.tile([8, 256], F16, tag="rec")
    with nc.allow_low_precision(reason="f16 softmax denom recip, tol 2e-2"):
        nc.vector.reciprocal(out=rec, in_=sums)
    hTn = p4h.tile([128, 4, 256], F16, tag="hTn")
    for cc in range(2):
        ps_rb = ps_r.tile([128, 2, 256], F32, tag="rb")
        for k in range(2):
            c = 2 * cc + k
            # broadcast rec rows into the partition layout of hT via PE:
            # sel_sb[:, c, :][s, p] = (s == 2c + p//64)
            nc.tensor.matmul(
                ps_rb[:, k, :], lhsT=sel_sb[:, c, :], rhs=rec, start=True, stop=True
            )
        for k in range(2):
            c = 2 * cc + k
            nc.vector.tensor_tensor(
                out=hTn[:, c, :], in0=hT[:, c, :], in1=ps_rb[:, k, :], op=ALU.mult
            )
    for it in range(2):
        ps = ps_y.tile([128, D], F32, tag="y")
        for c in range(4):
            nc.tensor.matmul(
                ps,
                lhsT=hTn[:, c, it * 128 : (it + 1) * 128],
                rhs=wout_sb[:, c, :],
                start=(c == 0),
                stop=(c == 3),
            )
        nc.vector.tensor_copy(out=y_sb[:, 2 * ci + it, :], in_=ps)


def _phase5_ln(nc, p5, y_sb, g_bc, eps_t, out_d, chunks):
    """Final LN over the given row chunks, batched (Sqrt table stays
    clear of the phase2 Exp stream)."""
    nch = len(chunks)
    mvs = p5.tile([128, 8, 2], F32, tag="mv5")
    for i, t in enumerate(chunks):
        stats = p5.tile([128, 6], F32, tag="st5")
        nc.vector.bn_stats(out=stats, in_=y_sb[:, t, :])
        nc.vector.bn_aggr(out=mvs[:, i, :], in_=stats)
    rstds = p5.tile([128, 8], F32, tag="rstd5")
    nc.scalar.activation(
        out=rstds[:, 0:nch], in_=mvs[:, 0:nch, 1], func=AF.Sqrt, bias=eps_t
    )
    nc.vector.reciprocal(out=rstds[:, 0:nch], in_=rstds[:, 0:nch])
    for i, t in enumerate(chunks):
        y_t = p5.tile([128, D], F32, tag="y5")
        nc.vector.tensor_scalar(
            out=y_t,
            in0=y_sb[:, t, :],
            scalar1=mvs[:, i, 0:1],
            scalar2=rstds[:, i : i + 1],
            op0=ALU.subtract,
            op1=ALU.mult,
        )
        nc.vector.tensor_tensor(out=y_t, in0=y_t, in1=g_bc, op=ALU.mult)
        nc.sync.dma_start(out=out_d[t * 128 : (t + 1) * 128, :], in_=y_t)


def build_attention_bass(n: int = N, b: int = B) -> bass.Bass:
    """Build the SPMD per-core Bass program (identical on all cores)."""
    rows = b * n
    assert rows % (NCORES * 128) == 0 and n % 512 == 0 and b == 4
    rows_pc = rows // NCORES
    n_ii = n // 512
    n_jc = n // 128

    nc = bacc.Bacc(num_devices=NCORES)

    x_d = nc.declare_dram_parameter("x", [rows, D], F16, isOutput=False)
    w_d = nc.declare_dram_parameter("w", [4, 128, 3 * DH], F16, isOutput=False)
    post_d = nc.declare_dram_parameter("post", [n, n], F16, isOutput=False)
    wout_d = nc.declare_dram_parameter("wout", [4, 128, D], F16, isOutput=False)
    g_d = nc.declare_dram_parameter("g", [1, D], F32, isOutput=False)
    sel_d = nc.declare_dram_parameter("sel", [4, 8, 128], F16, isOutput=False)
    out_d = nc.declare_dram_parameter("out", [rows_pc, D], F32, isOutput=True)

    a2a_ins = [
        nc.dram_tensor(f"a2a_in{ii}", [NCORES, DH + 1, 256], F16)
        for ii in range(n_ii)
    ]
    a2a_outs = [
        nc.dram_tensor(f"a2a_out{ii}", [NCORES, DH + 1, 256], F16)
        for ii in range(n_ii)
    ]
    warm_in = nc.dram_tensor("warm_in", [NCORES, 16], F16)
    warm_out = nc.dram_tensor("warm_out", [NCORES, 16], F16)

    groups = [list(range(NCORES))]

    with tile.TileContext(nc) as tc:
        with (
            tc.tile_pool(name="singles", bufs=1) as singles,
            tc.tile_pool(name="persist", bufs=1) as persist,
            tc.tile_pool(name="p2c", bufs=2) as p2c,
        ):
            # warm up the collective channels while phase1 computes
            nc.gpsimd.collective_compute(
                "AllToAll",
                ALU.bypass,
                replica_groups=groups,
                ins=[warm_in[:]],
                outs=[warm_out[:]],
            )
            ident = singles.tile([128, 128], F16)
            make_identity(nc, ident)
            eps_t = singles.tile([128, 1], F32)
            nc.vector.memset(eps_t, EPS)
            w_sb = singles.tile([128, 4, 3 * DH], F16)
            nc.sync.dma_start(out=w_sb, in_=w_d.rearrange("c p m -> p c m"))

            qT_sb = persist.tile([64, b, n], F16, name="qT")
            kT_sb = persist.tile([64, b, n], F16, name="kT")
            v_sb = persist.tile([128, b, n_jc, DH + 1], F16, name="v")
            nc.vector.memset(v_sb[:, :, :, DH : DH + 1], 1.0)

            # only the first j-chunks of pos[0] load before phase1 (keeps
            # the DMA queues clear for phase1's x tiles); the rest queue
            # right after phase1's emission, still well ahead of use
            pos_cur = _load_pos(nc, p2c, post_d, n, 0, js=range(0, 4))

            with (
                tc.tile_pool(name="p2", bufs=3) as p2,
                tc.tile_pool(name="p2o", bufs=2) as p2o,
                tc.tile_pool(name="p4", bufs=2) as p4,
                tc.tile_pool(name="p4s", bufs=1) as p4s,
                tc.tile_pool(name="p4h", bufs=2) as p4h,
                tc.tile_pool(name="p5", bufs=2) as p5,
            ):
                wout_sb = p4s.tile([128, 4, D], F16)
                nc.sync.dma_start(
                    out=wout_sb, in_=wout_d.rearrange("c p m -> p c m")
                )
                g_bc = p4s.tile([128, D], F32)
                nc.sync.dma_start(
                    out=g_bc, in_=g_d[0, :].partition_broadcast(128)
                )
                sel_sb = p4s.tile([8, 4, 128], F16)
                nc.sync.dma_start(
                    out=sel_sb, in_=sel_d.rearrange("c s p -> s c p")
                )
                y_sb = p4s.tile([128, 2 * n_ii, D], F32)
                P4LAG = 1
                _phase1(nc, tc, x_d, w_sb, eps_t, ident, qT_sb, kT_sb, v_sb,
                        n, b)
                _load_pos_rest(nc, pos_cur, post_d, n, 0, range(4, 16))
                with (
                    tc.tile_pool(name="ps_s", bufs=2, space="PSUM") as ps_s,
                    tc.tile_pool(name="ps_o", bufs=2, space="PSUM") as ps_o,
                    tc.tile_pool(name="ps_r", bufs=1, space="PSUM") as ps_r,
                    tc.tile_pool(name="ps_y", bufs=1, space="PSUM") as ps_y,
                ):
                    p2pools = (p2, p2o, ps_s, ps_o)
                    p4pools = (p4, p4h, ps_r, ps_y)
                    pos_tiles = {0: pos_cur, 1: _load_pos(
                        nc, p2c, post_d, n, 1)}
                    for ii in range(n_ii):
                        _phase2(
                            nc, p2pools, ident, qT_sb, kT_sb, v_sb, a2a_ins,
                            n, b, ii, pos_tiles.pop(ii),
                        )
                        nc.gpsimd.collective_compute(
                            "AllToAll",
                            ALU.bypass,
                            replica_groups=groups,
                            ins=[a2a_ins[ii][:]],
                            outs=[a2a_outs[ii][:]],
                        )
                        if ii >= P4LAG:
                            _phase4_proj(
                                nc, p4pools, a2a_outs[ii - P4LAG], wout_sb,
                                sel_sb, y_sb, ii - P4LAG,
                            )
                        if ii + 2 < n_ii:
                            # prefetch the after-next pos block here, after
                            # the collective trigger, so its DMA doesn't
                            # contend with the in-flight a2a transfer
                            pos_tiles[ii + 2] = _load_pos(
                                nc, p2c, post_d, n, ii + 2
                            )
                    # final LN for all but the last a2a chunk hides inside
                    # the last collective's transfer window
                    _phase5_ln(
                        nc, p5, y_sb, g_bc, eps_t, out_d,
                        list(range(2 * n_ii - 2)),
                    )
                    for ii in range(max(0, n_ii - P4LAG), n_ii):
                        _phase4_proj(
                            nc, p4pools, a2a_outs[ii], wout_sb, sel_sb,
                            y_sb, ii,
                        )
                    _phase5_ln(
                        nc, p5, y_sb, g_bc, eps_t, out_d,
                        [2 * n_ii - 2, 2 * n_ii - 1],
                    )

    nc.finalize()
    return nc


def make_in_maps(x, pos_bias, w_qkv, w_out, g_norm, g_out, n=N, b=B):
    """Host-side shard/layout prep: per-core input maps (no math beyond
    folding the LN gamma / attention scale diagonals into the weights)."""
    rows = b * n
    x16 = np.ascontiguousarray(x.reshape(rows, D)).astype(np.float16)
    w_eff = w_qkv * g_norm[:, None].astype(np.float32)
    wout16 = np.ascontiguousarray(w_out.reshape(4, 128, D)).astype(np.float16)
    g_row = np.ascontiguousarray(g_out.reshape(1, D)).astype(np.float32)
    sel = np.zeros((4, 8, 128), dtype=np.float16)
    for c in range(4):
        sel[c, 2 * c, 0:64] = 1.0
        sel[c, 2 * c + 1, 64:128] = 1.0
    hidden = HEADS * DH
    in_maps = []
    for h in range(NCORES):
        wq = w_eff[:, h * DH : (h + 1) * DH] * SCALE
        wk = w_eff[:, hidden + h * DH : hidden + (h + 1) * DH]
        wv = w_eff[:, 2 * hidden + h * DH : 2 * hidden + (h + 1) * DH]
        w_h = np.concatenate([wq, wk, wv], axis=1).reshape(4, 128, 3 * DH)
        posT = np.ascontiguousarray(pos_bias[h].T).astype(np.float16)
        in_maps.append(
            {
                "x": x16,
                "w": np.ascontiguousarray(w_h).astype(np.float16),
                "post": posT,
                "wout": wout16,
                "g": g_row,
                "sel": sel,
            }
        )
    return in_maps


def assemble_output(results, n=N, b=B):
    """Scatter per-core row shards back to the full [b, n, D] output."""
    out = np.empty((b, n, D), dtype=np.float32)
    n_ii = n // 512
    for c in range(NCORES):
        oc = results[c]["out"]
        bi = c // 2
        for ii in range(n_ii):
            i0 = 512 * ii + 256 * (c % 2)
            out[bi, i0 : i0 + 256, :] = oc[ii * 256 : (ii + 1) * 256, :]
    return out


_NC_CACHE: dict = {}


def _get_nc(n=N, b=B):
    key = (n, b)
    if key not in _NC_CACHE:
        _NC_CACHE[key] = build_attention_bass(n, b)
    return _NC_CACHE[key]


def kernel(x, pos_bias, w_qkv, w_out, g_norm, g_out, _trace=False):
    x = np.asarray(x, dtype=np.float32)
    pos_bias = np.asarray(pos_bias, dtype=np.float32)
    w_qkv = np.asarray(w_qkv, dtype=np.float32)
    w_out = np.asarray(w_out, dtype=np.float32)
    g_norm = np.asarray(g_norm, dtype=np.float32)
    g_out = np.asarray(g_out, dtype=np.float32)
    b, n, _ = x.shape

    nc = _get_nc(n, b)
    in_maps = make_in_maps(x, pos_bias, w_qkv, w_out, g_norm, g_out, n, b)
    res = run_bass_kernel_spmd(
        nc, in_maps, core_ids=list(range(NCORES)), trace=_trace
    )
    if _trace:
        kernel.last_results = res
    return assemble_output(res.results, n, b)


# revision 41
# speedup vs baseline: 1.0491x; 1.0162x over previous
"""Trainium2 Bass kernel for imagen-style self-attention with pos_bias.

Reference computation (fp32 jax):
    xn   = LN(x) * g_norm                      # gamma-only layernorm
    qkv  = xn @ w_qkv ; q,k,v per head (h=8, d=64) ; q *= d**-0.5
    sim  = q @ k^T + pos_bias[h]               # [b, h, n, n]
    attn = softmax(sim, -1)
    out  = LN((attn @ v) @ w_out) * g_out

Distribution: 8 cores, one head per core (tensor parallel over heads).
Each core computes LN+QKV projection for its head over the full batch,
full attention for its head, then AllToAlls (split into n/512 chunks so
comm overlaps compute) re-shard by sequence rows; each core runs the
output projection for a 1/8 row shard; the final LN runs batched at the
end (keeps the scalar engine's Exp activation table resident during the
whole attention phase).

Row shard mapping: block (b, ii) = rows (b, i in [512*ii, 512*ii+512))
is split into two 256-row halves h; half (b, ii, h) goes to core 2b+h.
So core c owns rows (b=c//2, i in [512*ii + 256*(c%2), +256)) for all
ii, in ii order.

Softmax denominators travel with the AllToAll payload (row DH of each
[DH+1, 256] chunk); the attention output is scaled by 2^-6 pre-cast so
unnormalized f16 values cannot overflow, and normalization happens on
the receiving side (scale cancels in o/sums).
"""

import numpy as np

import concourse.bass as bass
import concourse.bacc as bacc
import concourse.mybir as mybir
import concourse.tile as tile
from concourse.bass_utils import run_bass_kernel_spmd
from concourse.masks import make_identity

B = 4
N = 2048
D = 512
HEADS = 8
DH = 64
SCALE = DH**-0.5
EPS = 1e-5
NCORES = 8
OSCALE = 2.0**-6

F16 = mybir.dt.float16
F32 = mybir.dt.float32
AF = mybir.ActivationFunctionType
ALU = mybir.AluOpType


def _phase1(nc, tc, x_d, w_sb, eps_t, ident, qT_sb, kT_sb, v_sb, n, b):
    """LN + transpose + QKV projection over all rows."""
    rows = b * n
    n_spans = rows // 512
    with (
        tc.tile_pool(name="p1", bufs=3) as p1,
        tc.tile_pool(name="p1xT", bufs=2) as p1xT,
        tc.tile_pool(name="ps_t", bufs=2, space="PSUM") as ps_t,
        tc.tile_pool(name="ps_p", bufs=2, space="PSUM") as ps_p,
    ):
        for sp in range(n_spans):
            xnT = p1xT.tile([128, 4, 512], F16, tag="xnT")
            mvs = p1.tile([128, 4, 2], F32, tag="mvs")
            rstds = p1.tile([128, 4], F32, tag="rstds")
            x_ts = []
            for t in range(4):
                rt = sp * 4 + t
                x_t = p1.tile([128, D], F16, tag=f"x{t}", name=f"x{t}")
                x_ts.append(x_t)
                nc.sync.dma_start(out=x_t, in_=x_d[rt * 128 : (rt + 1) * 128, :])
                stats = p1.tile([128, 6], F32, tag="stats")
                nc.vector.bn_stats(out=stats, in_=x_t)
                nc.vector.bn_aggr(out=mvs[:, t, :], in_=stats)
            # batched rstd for the whole span: 1/sqrt(var + eps)
            nc.scalar.activation(
                out=rstds, in_=mvs[:, :, 1], func=AF.Sqrt, bias=eps_t
            )
            nc.vector.reciprocal(out=rstds, in_=rstds)
            for t in range(4):
                xn_t = p1.tile([128, D], F16, tag="xn")
                nc.vector.tensor_scalar(
                    out=xn_t,
                    in0=x_ts[t],
                    scalar1=mvs[:, t, 0:1],
                    scalar2=rstds[:, t : t + 1],
                    op0=ALU.subtract,
                    op1=ALU.mult,
                )
                # transpose 4 chunks into one psum bank, single evac cast
                ps = ps_t.tile([128, 512], F32, tag="tp")
                for c in range(4):
                    nc.tensor.matmul(
                        ps[:, c * 128 : (c + 1) * 128],
                        lhsT=xn_t[:, c * 128 : (c + 1) * 128],
                        rhs=ident,
                        start=(c == 0),
                        stop=(c == 3),
                    )
                nc.scalar.copy(
                    out=xnT[:, :, t * 128 : (t + 1) * 128],
                    in_=ps.rearrange("p (c w) -> p c w", c=4),
                )

            bi = (sp * 512) // n
            cols = slice((sp * 512) % n, (sp * 512) % n + 512)
            ps_q = ps_p.tile([64, 512], F32, tag="q")
            ps_k = ps_p.tile([64, 512], F32, tag="k")
            ps_v = ps_p.tile([64, 512], F32, tag="v")
            for c in range(4):
                st, fin = (c == 0), (c == 3)
                nc.tensor.matmul(
                    ps_q, lhsT=w_sb[:, c, 0:64], rhs=xnT[:, c, :], start=st, stop=fin
                )
                nc.tensor.matmul(
                    ps_k, lhsT=w_sb[:, c, 64:128], rhs=xnT[:, c, :], start=st, stop=fin
                )
                nc.tensor.matmul(
                    ps_v, lhsT=w_sb[:, c, 128:192], rhs=xnT[:, c, :], start=st, stop=fin
                )
            nc.vector.tensor_copy(out=qT_sb[:, bi, cols], in_=ps_q)
            nc.scalar.copy(out=kT_sb[:, bi, cols], in_=ps_k)
            vT_t = p1.tile([64, 512], F16, tag="vT")
            nc.scalar.copy(out=vT_t, in_=ps_v)
            # transpose vT [64, 512] into v natural [512, 64]: 4 transposes
            # into one psum bank, single strided evac
            jc0 = ((sp * 512) % n) // 128
            ps2 = ps_t.tile([128, 4, 64], F32, tag="tp", name="psv")
            for t in range(4):
                nc.tensor.matmul(
                    ps2[:, t, :],
                    lhsT=vT_t[:, t * 128 : (t + 1) * 128],
                    rhs=ident[0:64, 0:64],
                    start=(t == 0),
                    stop=(t == 3),
                )
            nc.scalar.copy(out=v_sb[:, bi, jc0 : jc0 + 4, 0:DH], in_=ps2)


def _load_pos(nc, p2c, post_d, n, ii, js=None):
    """Start DMA of the pos_bias column block for one i-span."""
    n_jc = n // 128
    pos_c = p2c.tile([128, n_jc, 512], F16, tag="posc")
    icols = slice(ii * 512, (ii + 1) * 512)
    for j in js if js is not None else range(n_jc):
        nc.sync.dma_start(
            out=pos_c[:, j, :], in_=post_d[j * 128 : (j + 1) * 128, icols]
        )
    return pos_c


def _load_pos_rest(nc, pos_c, post_d, n, ii, js):
    icols = slice(ii * 512, (ii + 1) * 512)
    for j in js:
        nc.sync.dma_start(
            out=pos_c[:, j, :], in_=post_d[j * 128 : (j + 1) * 128, icols]
        )


def _phase2(nc, pools, ident, qT_sb, kT_sb, v_sb, a2a_ins, n, b, ii, pos_c):
    """Attention for one i-span: S^T = pos^T + kT.T@qT ; exp ; O^T.

    The pos broadcast rides the PE (identity matmul into the S psum)
    deliberately: it keeps the PE the saturated rate-limiting engine,
    which holds the HAM clock gate open — scalar-bound variants let the
    PE idle per-pair and it gets throttled to 1.2 GHz, costing far more
    than the broadcast matmuls. j-chunk pairs share one [128, 2, 512] S
    psum (2 banks) so exp runs 1024 wide; O is left unnormalized
    (scaled by OSCALE) and the softmax sums row travels with the a2a
    payload."""
    n_jc = n // 128
    p2, p2o, ps_s, ps_o = pools
    icols = slice(ii * 512, (ii + 1) * 512)
    for bi in range(b):
        ps_O = ps_o.tile([DH + 1, 512], F32, tag="O")
        p_ts = {}

        def _o_mm(g):
            p_t = p_ts.pop(g)
            for h in range(2):
                j = 2 * g + h
                nc.tensor.matmul(
                    ps_O,
                    lhsT=v_sb[:, bi, j, :],
                    rhs=p_t[:, h, :],
                    start=(j == 0),
                    stop=(j == n_jc - 1),
                )

        for g in range(n_jc // 2):
            ps_S = ps_s.tile([128, 2, 512], F32, tag="S")
            p_t = p2.tile([128, 2, 512], F16, tag="P")
            p_ts[g] = p_t
            for h in range(2):
                nc.tensor.matmul(
                    ps_S[:, h, :],
                    lhsT=ident,
                    rhs=pos_c[:, 2 * g + h, :],
                    start=True,
                    stop=False,
                )
            for h in range(2):
                j = 2 * g + h
                nc.tensor.matmul(
                    ps_S[:, h, :],
                    lhsT=kT_sb[:, bi, j * 128 : (j + 1) * 128],
                    rhs=qT_sb[:, bi, icols],
                    start=False,
                    stop=True,
                )
            nc.scalar.activation(out=p_t, in_=ps_S, func=AF.Exp)
            if g >= 1:
                _o_mm(g - 1)
        _o_mm(n_jc // 2 - 1)
        o_t = p2o.tile([DH + 1, 512], F16, tag="onorm")
        nc.vector.tensor_scalar_mul(o_t, ps_O, OSCALE)
        for h in range(2):
            nc.sync.dma_start(
                out=a2a_ins[ii][2 * bi + h, :, :],
                in_=o_t[:, h * 256 : (h + 1) * 256],
            )


def _phase4_proj(nc, pools, src, wout_sb, sel_sb, y_sb, ci):
    """Out projection for one 256-row a2a chunk (normalize by sums row)."""
    p4, p4h, ps_r, ps_y = pools
    hT = p4h.tile([128, 4, 256], F16, tag="hT")
    for c in range(4):
        for two in range(2):
            nc.sync.dma_start(
                out=hT[two * DH : (two + 1) * DH, c, :],
                in_=src[2 * c + two, 0:DH, :],
            )
    sums = p4.tile([8, 256], F16, tag="sums")
    nc.sync.dma_start(out=sums, in_=src[:, DH, :])
    rec = p4.tile([8, 256], F16, tag="rec")
    with nc.allow_low_precision(reason="f16 softmax denom recip, tol 2e-2"):
        nc.vector.reciprocal(out=rec, in_=sums)
    hTn = p4h.tile([128, 4, 256], F16, tag="hTn")
    for cc in range(2):
        ps_rb = ps_r.tile([128, 2, 256], F32, tag="rb")
        for k in range(2):
            c = 2 * cc + k
            # broadcast rec rows into the partition layout of hT via PE:
            # sel_sb[:, c, :][s, p] = (s == 2c + p//64)
            nc.tensor.matmul(
                ps_rb[:, k, :], lhsT=sel_sb[:, c, :], rhs=rec, start=True, stop=True
            )
        for k in range(2):
            c = 2 * cc + k
            nc.vector.tensor_tensor(
                out=hTn[:, c, :], in0=hT[:, c, :], in1=ps_rb[:, k, :], op=ALU.mult
            )
    for it in range(2):
        ps = ps_y.tile([128, D], F32, tag="y")
        for c in range(4):
            nc.tensor.matmul(
                ps,
                lhsT=hTn[:, c, it * 128 : (it + 1) * 128],
                rhs=wout_sb[:, c, :],
                start=(c == 0),
                stop=(c == 3),
            )
        nc.vector.tensor_copy(out=y_sb[:, 2 * ci + it, :], in_=ps)


def _phase5_ln(nc, p5, y_sb, g_bc, eps_t, out_d, chunks):
    """Final LN over the given row chunks, batched (Sqrt table stays
    clear of the phase2 Exp stream)."""
    nch = len(chunks)
    mvs = p5.tile([128, 8, 2], F32, tag="mv5")
    for i, t in enumerate(chunks):
        stats = p5.tile([128, 6], F32, tag="st5")
        nc.vector.bn_stats(out=stats, in_=y_sb[:, t, :])
        nc.vector.bn_aggr(out=mvs[:, i, :], in_=stats)
    rstds = p5.tile([128, 8], F32, tag="rstd5")
    nc.scalar.activation(
        out=rstds[:, 0:nch], in_=mvs[:, 0:nch, 1], func=AF.Sqrt, bias=eps_t
    )
    nc.vector.reciprocal(out=rstds[:, 0:nch], in_=rstds[:, 0:nch])
    for i, t in enumerate(chunks):
        y_t = p5.tile([128, D], F32, tag="y5")
        nc.vector.tensor_scalar(
            out=y_t,
            in0=y_sb[:, t, :],
            scalar1=mvs[:, i, 0:1],
            scalar2=rstds[:, i : i + 1],
            op0=ALU.subtract,
            op1=ALU.mult,
        )
        nc.vector.tensor_tensor(out=y_t, in0=y_t, in1=g_bc, op=ALU.mult)
        nc.sync.dma_start(out=out_d[t * 128 : (t + 1) * 128, :], in_=y_t)


def build_attention_bass(n: int = N, b: int = B) -> bass.Bass:
    """Build the SPMD per-core Bass program (identical on all cores)."""
    rows = b * n
    assert rows % (NCORES * 128) == 0 and n % 512 == 0 and b == 4
    rows_pc = rows // NCORES
    n_ii = n // 512
    n_jc = n // 128

    nc = bacc.Bacc(num_devices=NCORES)

    x_d = nc.declare_dram_parameter("x", [rows, D], F16, isOutput=False)
    w_d = nc.declare_dram_parameter("w", [4, 128, 3 * DH], F16, isOutput=False)
    post_d = nc.declare_dram_parameter("post", [n, n], F16, isOutput=False)
    wout_d = nc.declare_dram_parameter("wout", [4, 128, D], F16, isOutput=False)
    g_d = nc.declare_dram_parameter("g", [1, D], F32, isOutput=False)
    sel_d = nc.declare_dram_parameter("sel", [4, 8, 128], F16, isOutput=False)
    out_d = nc.declare_dram_parameter("out", [rows_pc, D], F32, isOutput=True)

    a2a_ins = [
        nc.dram_tensor(f"a2a_in{ii}", [NCORES, DH + 1, 256], F16)
        for ii in range(n_ii)
    ]
    a2a_outs = [
        nc.dram_tensor(f"a2a_out{ii}", [NCORES, DH + 1, 256], F16)
        for ii in range(n_ii)
    ]
    warm_in = nc.dram_tensor("warm_in", [NCORES, 16], F16)
    warm_out = nc.dram_tensor("warm_out", [NCORES, 16], F16)

    groups = [list(range(NCORES))]

    with tile.TileContext(nc) as tc:
        with (
            tc.tile_pool(name="singles", bufs=1) as singles,
            tc.tile_pool(name="persist", bufs=1) as persist,
            tc.tile_pool(name="p2c", bufs=2) as p2c,
        ):
            # warm up the collective channels while phase1 computes
            nc.gpsimd.collective_compute(
                "AllToAll",
                ALU.bypass,
                replica_groups=groups,
                ins=[warm_in[:]],
                outs=[warm_out[:]],
            )
            ident = singles.tile([128, 128], F16)
            make_identity(nc, ident)
            eps_t = singles.tile([128, 1], F32)
            nc.vector.memset(eps_t, EPS)
            w_sb = singles.tile([128, 4, 3 * DH], F16)
            nc.sync.dma_start(out=w_sb, in_=w_d.rearrange("c p m -> p c m"))

            qT_sb = persist.tile([64, b, n], F16, name="qT")
            kT_sb = persist.tile([64, b, n], F16, name="kT")
            v_sb = persist.tile([128, b, n_jc, DH + 1], F16, name="v")
            nc.vector.memset(v_sb[:, :, :, DH : DH + 1], 1.0)

            # only the first j-chunks of pos[0] load before phase1 (keeps
            # the DMA queues clear for phase1's x tiles); the rest queue
            # right after phase1's emission, still well ahead of use
            pos_cur = _load_pos(nc, p2c, post_d, n, 0, js=range(0, 4))

            with (
                tc.tile_pool(name="p2", bufs=3) as p2,
                tc.tile_pool(name="p2o", bufs=2) as p2o,
                tc.tile_pool(name="p4", bufs=2) as p4,
                tc.tile_pool(name="p4s", bufs=1) as p4s,
                tc.tile_pool(name="p4h", bufs=2) as p4h,
                tc.tile_pool(name="p5", bufs=2) as p5,
            ):
                wout_sb = p4s.tile([128, 4, D], F16)
                nc.sync.dma_start(
                    out=wout_sb, in_=wout_d.rearrange("c p m -> p c m")
                )
                g_bc = p4s.tile([128, D], F32)
                nc.sync.dma_start(
                    out=g_bc, in_=g_d[0, :].partition_broadcast(128)
                )
                sel_sb = p4s.tile([8, 4, 128], F16)
                nc.sync.dma_start(
                    out=sel_sb, in_=sel_d.rearrange("c s p -> s c p")
                )
                y_sb = p4s.tile([128, 2 * n_ii, D], F32)
                P4LAG = 1
                _phase1(nc, tc, x_d, w_sb, eps_t, ident, qT_sb, kT_sb, v_sb,
                        n, b)
                _load_pos_rest(nc, pos_cur, post_d, n, 0, range(4, 16))
                with (
                    tc.tile_pool(name="ps_s", bufs=2, space="PSUM") as ps_s,
                    tc.tile_pool(name="ps_o", bufs=2, space="PSUM") as ps_o,
                    tc.tile_pool(name="ps_r", bufs=1, space="PSUM") as ps_r,
                    tc.tile_pool(name="ps_y", bufs=1, space="PSUM") as ps_y,
                ):
                    p2pools = (p2, p2o, ps_s, ps_o)
                    p4pools = (p4, p4h, ps_r, ps_y)
                    pos_tiles = {0: pos_cur, 1: _load_pos(
                        nc, p2c, post_d, n, 1)}
                    for ii in range(n_ii):
                        _phase2(
                            nc, p2pools, ident, qT_sb, kT_sb, v_sb, a2a_ins,
                            n, b, ii, pos_tiles.pop(ii),
                        )
                        nc.gpsimd.collective_compute(
                            "AllToAll",
                            ALU.bypass,
                            replica_groups=groups,
                            ins=[a2a_ins[ii][:]],
                            outs=[a2a_outs[ii][:]],
                        )
                        if ii >= P4LAG:
                            _phase4_proj(
                                nc, p4pools, a2a_outs[ii - P4LAG], wout_sb,
                                sel_sb, y_sb, ii - P4LAG,
                            )
                        if ii + 2 < n_ii:
                            # prefetch the after-next pos block here, after
                            # the collective trigger, so its DMA doesn't
                            # contend with the in-flight a2a transfer
                            pos_tiles[ii + 2] = _load_pos(
                                nc, p2c, post_d, n, ii + 2
                            )
                    # final LN for all but the last a2a chunk hides inside
                    # the last collective's transfer window
                    _phase5_ln(
                        nc, p5, y_sb, g_bc, eps_t, out_d,
                        list(range(2 * n_ii - 2)),
                    )
                    for ii in range(max(0, n_ii - P4LAG), n_ii):
                        _phase4_proj(
                            nc, p4pools, a2a_outs[ii], wout_sb, sel_sb,
                            y_sb, ii,
                        )
                    _phase5_ln(
                        nc, p5, y_sb, g_bc, eps_t, out_d,
                        [2 * n_ii - 2, 2 * n_ii - 1],
                    )

    nc.finalize()
    return nc


def make_in_maps(x, pos_bias, w_qkv, w_out, g_norm, g_out, n=N, b=B):
    """Host-side shard/layout prep: per-core input maps (no math beyond
    folding the LN gamma / attention scale diagonals into the weights)."""
    rows = b * n
    x16 = np.ascontiguousarray(x.reshape(rows, D)).astype(np.float16)
    w_eff = w_qkv * g_norm[:, None].astype(np.float32)
    wout16 = np.ascontiguousarray(w_out.reshape(4, 128, D)).astype(np.float16)
    g_row = np.ascontiguousarray(g_out.reshape(1, D)).astype(np.float32)
    sel = np.zeros((4, 8, 128), dtype=np.float16)
    for c in range(4):
        sel[c, 2 * c, 0:64] = 1.0
        sel[c, 2 * c + 1, 64:128] = 1.0
    hidden = HEADS * DH
    in_maps = []
    for h in range(NCORES):
        wq = w_eff[:, h * DH : (h + 1) * DH] * SCALE
        wk = w_eff[:, hidden + h * DH : hidden + (h + 1) * DH]
        wv = w_eff[:, 2 * hidden + h * DH : 2 * hidden + (h + 1) * DH]
        w_h = np.concatenate([wq, wk, wv], axis=1).reshape(4, 128, 3 * DH)
        posT = np.ascontiguousarray(pos_bias[h].T).astype(np.float16)
        in_maps.append(
            {
                "x": x16,
                "w": np.ascontiguousarray(w_h).astype(np.float16),
                "post": posT,
                "wout": wout16,
                "g": g_row,
                "sel": sel,
            }
        )
    return in_maps


def assemble_output(results, n=N, b=B):
    """Scatter per-core row shards back to the full [b, n, D] output."""
    out = np.empty((b, n, D), dtype=np.float32)
    n_ii = n // 512
    for c in range(NCORES):
        oc = results[c]["out"]
        bi = c // 2
        for ii in range(n_ii):
            i0 = 512 * ii + 256 * (c % 2)
            out[bi, i0 : i0 + 256, :] = oc[ii * 256 : (ii + 1) * 256, :]
    return out


_NC_CACHE: dict = {}


def _get_nc(n=N, b=B):
    key = (n, b)
    if key not in _NC_CACHE:
        _NC_CACHE[key] = build_attention_bass(n, b)
    return _NC_CACHE[key]


def kernel(x, pos_bias, w_qkv, w_out, g_norm, g_out, _trace=False):
    x = np.asarray(x, dtype=np.float32)
    pos_bias = np.asarray(pos_bias, dtype=np.float32)
    w_qkv = np.asarray(w_qkv, dtype=np.float32)
    w_out = np.asarray(w_out, dtype=np.float32)
    g_norm = np.asarray(g_norm, dtype=np.float32)
    g_out = np.asarray(g_out, dtype=np.float32)
    b, n, _ = x.shape

    nc = _get_nc(n, b)
    in_maps = make_in_maps(x, pos_bias, w_qkv, w_out, g_norm, g_out, n, b)
    res = run_bass_kernel_spmd(
        nc, in_maps, core_ids=list(range(NCORES)), trace=_trace
    )
    if _trace:
        kernel.last_results = res
    return assemble_output(res.results, n, b)


# revision 42
# speedup vs baseline: 1.0759x; 1.0255x over previous
"""Trainium2 Bass kernel for imagen-style self-attention with pos_bias.

Reference computation (fp32 jax):
    xn   = LN(x) * g_norm                      # gamma-only layernorm
    qkv  = xn @ w_qkv ; q,k,v per head (h=8, d=64) ; q *= d**-0.5
    sim  = q @ k^T + pos_bias[h]               # [b, h, n, n]
    attn = softmax(sim, -1)
    out  = LN((attn @ v) @ w_out) * g_out

Distribution: 8 cores, one head per core (tensor parallel over heads).
Each core computes LN+QKV projection for its head over the full batch,
full attention for its head, then AllToAlls (split into n/512 chunks so
comm overlaps compute) re-shard by sequence rows; each core runs the
output projection for a 1/8 row shard; the final LN runs batched at the
end (keeps the scalar engine's Exp activation table resident during the
whole attention phase).

Row shard mapping: block (b, ii) = rows (b, i in [512*ii, 512*ii+512))
is split into two 256-row halves h; half (b, ii, h) goes to core 2b+h.
So core c owns rows (b=c//2, i in [512*ii + 256*(c%2), +256)) for all
ii, in ii order.

Softmax denominators travel with the AllToAll payload (row DH of each
[DH+1, 256] chunk); the attention output is scaled by 2^-6 pre-cast so
unnormalized f16 values cannot overflow, and normalization happens on
the receiving side (scale cancels in o/sums).
"""

import numpy as np

import concourse.bass as bass
import concourse.bacc as bacc
import concourse.mybir as mybir
import concourse.tile as tile
from concourse.bass_utils import run_bass_kernel_spmd
from concourse.masks import make_identity

B = 4
N = 2048
D = 512
HEADS = 8
DH = 64
SCALE = DH**-0.5
EPS = 1e-5
NCORES = 8
OSCALE = 2.0**-6

F16 = mybir.dt.float16
F32 = mybir.dt.float32
AF = mybir.ActivationFunctionType
ALU = mybir.AluOpType


def _phase1(nc, tc, x_d, w_sb, eps_t, ident, qT_sb, kT_sb, v_sb, n, b):
    """LN + transpose + QKV projection over all rows."""
    rows = b * n
    n_spans = rows // 512
    with (
        tc.tile_pool(name="p1", bufs=3) as p1,
        tc.tile_pool(name="p1xT", bufs=2) as p1xT,
        tc.tile_pool(name="ps_t", bufs=2, space="PSUM") as ps_t,
        tc.tile_pool(name="ps_p", bufs=2, space="PSUM") as ps_p,
    ):
        for sp in range(n_spans):
            xnT = p1xT.tile([128, 4, 512], F16, tag="xnT")
            mvs = p1.tile([128, 4, 2], F32, tag="mvs")
            rstds = p1.tile([128, 4], F32, tag="rstds")
            x_sp = p1.tile([128, 4, D], F16, tag="xsp")
            nc.sync.dma_start(
                out=x_sp,
                in_=x_d.rearrange("(q p) d -> p q d", p=128)[
                    :, sp * 4 : sp * 4 + 4, :
                ],
            )
            for t in range(4):
                stats = p1.tile([128, 6], F32, tag="stats")
                nc.vector.bn_stats(out=stats, in_=x_sp[:, t, :])
                nc.vector.bn_aggr(out=mvs[:, t, :], in_=stats)
            # batched rstd for the whole span: 1/sqrt(var + eps)
            nc.scalar.activation(
                out=rstds, in_=mvs[:, :, 1], func=AF.Sqrt, bias=eps_t
            )
            nc.vector.reciprocal(out=rstds, in_=rstds)
            for t in range(4):
                xn_t = p1.tile([128, D], F16, tag="xn")
                nc.vector.tensor_scalar(
                    out=xn_t,
                    in0=x_sp[:, t, :],
                    scalar1=mvs[:, t, 0:1],
                    scalar2=rstds[:, t : t + 1],
                    op0=ALU.subtract,
                    op1=ALU.mult,
                )
                # transpose 4 chunks into one psum bank, single evac cast
                ps = ps_t.tile([128, 512], F32, tag="tp")
                for c in range(4):
                    nc.tensor.matmul(
                        ps[:, c * 128 : (c + 1) * 128],
                        lhsT=xn_t[:, c * 128 : (c + 1) * 128],
                        rhs=ident,
                        start=(c == 0),
                        stop=(c == 3),
                    )
                nc.scalar.copy(
                    out=xnT[:, :, t * 128 : (t + 1) * 128],
                    in_=ps.rearrange("p (c w) -> p c w", c=4),
                )

            bi = (sp * 512) // n
            cols = slice((sp * 512) % n, (sp * 512) % n + 512)
            ps_q = ps_p.tile([64, 512], F32, tag="q")
            ps_k = ps_p.tile([64, 512], F32, tag="k")
            ps_v = ps_p.tile([64, 512], F32, tag="v")
            for c in range(4):
                st, fin = (c == 0), (c == 3)
                nc.tensor.matmul(
                    ps_q, lhsT=w_sb[:, c, 0:64], rhs=xnT[:, c, :], start=st, stop=fin
                )
                nc.tensor.matmul(
                    ps_k, lhsT=w_sb[:, c, 64:128], rhs=xnT[:, c, :], start=st, stop=fin
                )
                nc.tensor.matmul(
                    ps_v, lhsT=w_sb[:, c, 128:192], rhs=xnT[:, c, :], start=st, stop=fin
                )
            nc.vector.tensor_copy(out=qT_sb[:, bi, cols], in_=ps_q)
            nc.scalar.copy(out=kT_sb[:, bi, cols], in_=ps_k)
            vT_t = p1.tile([64, 512], F16, tag="vT")
            nc.scalar.copy(out=vT_t, in_=ps_v)
            # transpose vT [64, 512] into v natural [512, 64]: 4 transposes
            # into one psum bank, single strided evac
            jc0 = ((sp * 512) % n) // 128
            ps2 = ps_t.tile([128, 4, 64], F32, tag="tp", name="psv")
            for t in range(4):
                nc.tensor.matmul(
                    ps2[:, t, :],
                    lhsT=vT_t[:, t * 128 : (t + 1) * 128],
                    rhs=ident[0:64, 0:64],
                    start=(t == 0),
                    stop=(t == 3),
                )
            nc.scalar.copy(out=v_sb[:, bi, jc0 : jc0 + 4, 0:DH], in_=ps2)


def _load_pos(nc, p2c, post_d, n, ii, js=None):
    """Start DMA of the pos_bias column block for one i-span."""
    n_jc = n // 128
    pos_c = p2c.tile([128, n_jc, 512], F16, tag="posc")
    icols = slice(ii * 512, (ii + 1) * 512)
    srcv = post_d.rearrange("(j p) m -> p j m", p=128)
    if js is None:
        js = range(n_jc)
    nc.sync.dma_start(
        out=pos_c[:, js.start : js.stop, :],
        in_=srcv[:, js.start : js.stop, icols],
    )
    return pos_c


def _load_pos_rest(nc, pos_c, post_d, n, ii, js):
    icols = slice(ii * 512, (ii + 1) * 512)
    srcv = post_d.rearrange("(j p) m -> p j m", p=128)
    nc.sync.dma_start(
        out=pos_c[:, js.start : js.stop, :],
        in_=srcv[:, js.start : js.stop, icols],
    )


def _phase2(nc, pools, ident, qT_sb, kT_sb, v_sb, a2a_ins, n, b, ii, pos_c):
    """Attention for one i-span: S^T = pos^T + kT.T@qT ; exp ; O^T.

    The pos broadcast rides the PE (identity matmul into the S psum)
    deliberately: it keeps the PE the saturated rate-limiting engine,
    which holds the HAM clock gate open — scalar-bound variants let the
    PE idle per-pair and it gets throttled to 1.2 GHz, costing far more
    than the broadcast matmuls. j-chunk pairs share one [128, 2, 512] S
    psum (2 banks) so exp runs 1024 wide; O is left unnormalized
    (scaled by OSCALE) and the softmax sums row travels with the a2a
    payload."""
    n_jc = n // 128
    p2, p2o, ps_s, ps_o = pools
    icols = slice(ii * 512, (ii + 1) * 512)
    for bi in range(b):
        ps_O = ps_o.tile([DH + 1, 512], F32, tag="O")
        p_ts = {}

        def _o_mm(g):
            p_t = p_ts.pop(g)
            for h in range(2):
                j = 2 * g + h
                nc.tensor.matmul(
                    ps_O,
                    lhsT=v_sb[:, bi, j, :],
                    rhs=p_t[:, h, :],
                    start=(j == 0),
                    stop=(j == n_jc - 1),
                )

        for g in range(n_jc // 2):
            ps_S = ps_s.tile([128, 2, 512], F32, tag="S")
            p_t = p2.tile([128, 2, 512], F16, tag="P")
            p_ts[g] = p_t
            for h in range(2):
                nc.tensor.matmul(
                    ps_S[:, h, :],
                    lhsT=ident,
                    rhs=pos_c[:, 2 * g + h, :],
                    start=True,
                    stop=False,
                )
            for h in range(2):
                j = 2 * g + h
                nc.tensor.matmul(
                    ps_S[:, h, :],
                    lhsT=kT_sb[:, bi, j * 128 : (j + 1) * 128],
                    rhs=qT_sb[:, bi, icols],
                    start=False,
                    stop=True,
                )
            nc.scalar.activation(out=p_t, in_=ps_S, func=AF.Exp)
            if g >= 1:
                _o_mm(g - 1)
        _o_mm(n_jc // 2 - 1)
        o_t = p2o.tile([DH + 1, 512], F16, tag="onorm")
        nc.vector.tensor_scalar_mul(o_t, ps_O, OSCALE)
        for h in range(2):
            nc.sync.dma_start(
                out=a2a_ins[ii][2 * bi + h, :, :],
                in_=o_t[:, h * 256 : (h + 1) * 256],
            )


def _phase4_proj(nc, pools, src, wout_sb, sel_sb, y_sb, ci):
    """Out projection for one 256-row a2a chunk (normalize by sums row)."""
    p4, p4h, ps_r, ps_y = pools
    hT = p4h.tile([128, 4, 256], F16, tag="hT")
    for c in range(4):
        for two in range(2):
            nc.sync.dma_start(
                out=hT[two * DH : (two + 1) * DH, c, :],
                in_=src[2 * c + two, 0:DH, :],
            )
    sums = p4.tile([8, 256], F16, tag="sums")
    nc.sync.dma_start(out=sums, in_=src[:, DH, :])
    rec = p4.tile([8, 256], F16, tag="rec")
    with nc.allow_low_precision(reason="f16 softmax denom recip, tol 2e-2"):
        nc.vector.reciprocal(out=rec, in_=sums)
    hTn = p4h.tile([128, 4, 256], F16, tag="hTn")
    for cc in range(2):
        ps_rb = ps_r.tile([128, 2, 256], F32, tag="rb")
        for k in range(2):
            c = 2 * cc + k
            # broadcast rec rows into the partition layout of hT via PE:
            # sel_sb[:, c, :][s, p] = (s == 2c + p//64)
            nc.tensor.matmul(
                ps_rb[:, k, :], lhsT=sel_sb[:, c, :], rhs=rec, start=True, stop=True
            )
        for k in range(2):
            c = 2 * cc + k
            nc.vector.tensor_tensor(
                out=hTn[:, c, :], in0=hT[:, c, :], in1=ps_rb[:, k, :], op=ALU.mult
            )
    for it in range(2):
        ps = ps_y.tile([128, D], F32, tag="y")
        for c in range(4):
            nc.tensor.matmul(
                ps,
                lhsT=hTn[:, c, it * 128 : (it + 1) * 128],
                rhs=wout_sb[:, c, :],
                start=(c == 0),
                stop=(c == 3),
            )
        nc.vector.tensor_copy(out=y_sb[:, 2 * ci + it, :], in_=ps)


def _phase5_ln(nc, p5, y_sb, g_bc, eps_t, out_d, chunks):
    """Final LN over the given row chunks, batched (Sqrt table stays
    clear of the phase2 Exp stream)."""
    nch = len(chunks)
    mvs = p5.tile([128, 8, 2], F32, tag="mv5")
    for i, t in enumerate(chunks):
        stats = p5.tile([128, 6], F32, tag="st5")
        nc.vector.bn_stats(out=stats, in_=y_sb[:, t, :])
        nc.vector.bn_aggr(out=mvs[:, i, :], in_=stats)
    rstds = p5.tile([128, 8], F32, tag="rstd5")
    nc.scalar.activation(
        out=rstds[:, 0:nch], in_=mvs[:, 0:nch, 1], func=AF.Sqrt, bias=eps_t
    )
    nc.vector.reciprocal(out=rstds[:, 0:nch], in_=rstds[:, 0:nch])
    for i, t in enumerate(chunks):
        y_t = p5.tile([128, D], F32, tag="y5")
        nc.vector.tensor_scalar(
            out=y_t,
            in0=y_sb[:, t, :],
            scalar1=mvs[:, i, 0:1],
            scalar2=rstds[:, i : i + 1],
            op0=ALU.subtract,
            op1=ALU.mult,
        )
        nc.vector.tensor_tensor(out=y_t, in0=y_t, in1=g_bc, op=ALU.mult)
        nc.sync.dma_start(out=out_d[t * 128 : (t + 1) * 128, :], in_=y_t)


def build_attention_bass(n: int = N, b: int = B) -> bass.Bass:
    """Build the SPMD per-core Bass program (identical on all cores)."""
    rows = b * n
    assert rows % (NCORES * 128) == 0 and n % 512 == 0 and b == 4
    rows_pc = rows // NCORES
    n_ii = n // 512
    n_jc = n // 128

    nc = bacc.Bacc(num_devices=NCORES)

    x_d = nc.declare_dram_parameter("x", [rows, D], F16, isOutput=False)
    w_d = nc.declare_dram_parameter("w", [4, 128, 3 * DH], F16, isOutput=False)
    post_d = nc.declare_dram_parameter("post", [n, n], F16, isOutput=False)
    wout_d = nc.declare_dram_parameter("wout", [4, 128, D], F16, isOutput=False)
    g_d = nc.declare_dram_parameter("g", [1, D], F32, isOutput=False)
    sel_d = nc.declare_dram_parameter("sel", [4, 8, 128], F16, isOutput=False)
    out_d = nc.declare_dram_parameter("out", [rows_pc, D], F32, isOutput=True)

    a2a_ins = [
        nc.dram_tensor(f"a2a_in{ii}", [NCORES, DH + 1, 256], F16)
        for ii in range(n_ii)
    ]
    a2a_outs = [
        nc.dram_tensor(f"a2a_out{ii}", [NCORES, DH + 1, 256], F16)
        for ii in range(n_ii)
    ]
    warm_in = nc.dram_tensor("warm_in", [NCORES, 16], F16)
    warm_out = nc.dram_tensor("warm_out", [NCORES, 16], F16)

    groups = [list(range(NCORES))]

    with tile.TileContext(nc) as tc:
        with (
            tc.tile_pool(name="singles", bufs=1) as singles,
            tc.tile_pool(name="persist", bufs=1) as persist,
            tc.tile_pool(name="p2c", bufs=2) as p2c,
        ):
            # warm up the collective channels while phase1 computes
            nc.gpsimd.collective_compute(
                "AllToAll",
                ALU.bypass,
                replica_groups=groups,
                ins=[warm_in[:]],
                outs=[warm_out[:]],
            )
            ident = singles.tile([128, 128], F16)
            make_identity(nc, ident)
            eps_t = singles.tile([128, 1], F32)
            nc.vector.memset(eps_t, EPS)
            w_sb = singles.tile([128, 4, 3 * DH], F16)
            nc.sync.dma_start(out=w_sb, in_=w_d.rearrange("c p m -> p c m"))

            qT_sb = persist.tile([64, b, n], F16, name="qT")
            kT_sb = persist.tile([64, b, n], F16, name="kT")
            v_sb = persist.tile([128, b, n_jc, DH + 1], F16, name="v")
            nc.vector.memset(v_sb[:, :, :, DH : DH + 1], 1.0)

            # only the first j-chunks of pos[0] load before phase1 (keeps
            # the DMA queues clear for phase1's x tiles); the rest queue
            # right after phase1's emission, still well ahead of use
            pos_cur = _load_pos(nc, p2c, post_d, n, 0, js=range(0, 4))

            with (
                tc.tile_pool(name="p2", bufs=3) as p2,
                tc.tile_pool(name="p2o", bufs=2) as p2o,
                tc.tile_pool(name="p4", bufs=2) as p4,
                tc.tile_pool(name="p4s", bufs=1) as p4s,
                tc.tile_pool(name="p4h", bufs=2) as p4h,
                tc.tile_pool(name="p5", bufs=2) as p5,
            ):
                wout_sb = p4s.tile([128, 4, D], F16)
                nc.sync.dma_start(
                    out=wout_sb, in_=wout_d.rearrange("c p m -> p c m")
                )
                g_bc = p4s.tile([128, D], F32)
                nc.sync.dma_start(
                    out=g_bc, in_=g_d[0, :].partition_broadcast(128)
                )
                sel_sb = p4s.tile([8, 4, 128], F16)
                nc.sync.dma_start(
                    out=sel_sb, in_=sel_d.rearrange("c s p -> s c p")
                )
                y_sb = p4s.tile([128, 2 * n_ii, D], F32)
                P4LAG = 1
                _phase1(nc, tc, x_d, w_sb, eps_t, ident, qT_sb, kT_sb, v_sb,
                        n, b)
                _load_pos_rest(nc, pos_cur, post_d, n, 0, range(4, 16))
                with (
                    tc.tile_pool(name="ps_s", bufs=2, space="PSUM") as ps_s,
                    tc.tile_pool(name="ps_o", bufs=2, space="PSUM") as ps_o,
                    tc.tile_pool(name="ps_r", bufs=1, space="PSUM") as ps_r,
                    tc.tile_pool(name="ps_y", bufs=1, space="PSUM") as ps_y,
                ):
                    p2pools = (p2, p2o, ps_s, ps_o)
                    p4pools = (p4, p4h, ps_r, ps_y)
                    pos_tiles = {0: pos_cur, 1: _load_pos(
                        nc, p2c, post_d, n, 1)}
                    for ii in range(n_ii):
                        _phase2(
                            nc, p2pools, ident, qT_sb, kT_sb, v_sb, a2a_ins,
                            n, b, ii, pos_tiles.pop(ii),
                        )
                        nc.gpsimd.collective_compute(
                            "AllToAll",
                            ALU.bypass,
                            replica_groups=groups,
                            ins=[a2a_ins[ii][:]],
                            outs=[a2a_outs[ii][:]],
                        )
                        if ii >= P4LAG:
                            _phase4_proj(
                                nc, p4pools, a2a_outs[ii - P4LAG], wout_sb,
                                sel_sb, y_sb, ii - P4LAG,
                            )
                        if ii + 2 < n_ii:
                            # prefetch the after-next pos block here, after
                            # the collective trigger, so its DMA doesn't
                            # contend with the in-flight a2a transfer
                            pos_tiles[ii + 2] = _load_pos(
                                nc, p2c, post_d, n, ii + 2
                            )
                    # final LN for all but the last a2a chunk hides inside
                    # the last collective's transfer window
                    _phase5_ln(
                        nc, p5, y_sb, g_bc, eps_t, out_d,
                        list(range(2 * n_ii - 2)),
                    )
                    for ii in range(max(0, n_ii - P4LAG), n_ii):
                        _phase4_proj(
                            nc, p4pools, a2a_outs[ii], wout_sb, sel_sb,
                            y_sb, ii,
                        )
                    _phase5_ln(
                        nc, p5, y_sb, g_bc, eps_t, out_d,
                        [2 * n_ii - 2, 2 * n_ii - 1],
                    )

    nc.finalize()
    return nc


def make_in_maps(x, pos_bias, w_qkv, w_out, g_norm, g_out, n=N, b=B):
    """Host-side shard/layout prep: per-core input maps (no math beyond
    folding the LN gamma / attention scale diagonals into the weights)."""
    rows = b * n
    x16 = np.ascontiguousarray(x.reshape(rows, D)).astype(np.float16)
    w_eff = w_qkv * g_norm[:, None].astype(np.float32)
    wout16 = np.ascontiguousarray(w_out.reshape(4, 128, D)).astype(np.float16)
    g_row = np.ascontiguousarray(g_out.reshape(1, D)).astype(np.float32)
    sel = np.zeros((4, 8, 128), dtype=np.float16)
    for c in range(4):
        sel[c, 2 * c, 0:64] = 1.0
        sel[c, 2 * c + 1, 64:128] = 1.0
    hidden = HEADS * DH
    in_maps = []
    for h in range(NCORES):
        wq = w_eff[:, h * DH : (h + 1) * DH] * SCALE
        wk = w_eff[:, hidden + h * DH : hidden + (h + 1) * DH]
        wv = w_eff[:, 2 * hidden + h * DH : 2 * hidden + (h + 1) * DH]
        w_h = np.concatenate([wq, wk, wv], axis=1).reshape(4, 128, 3 * DH)
        posT = np.ascontiguousarray(pos_bias[h].T).astype(np.float16)
        in_maps.append(
            {
                "x": x16,
                "w": np.ascontiguousarray(w_h).astype(np.float16),
                "post": posT,
                "wout": wout16,
                "g": g_row,
                "sel": sel,
            }
        )
    return in_maps


def assemble_output(results, n=N, b=B):
    """Scatter per-core row shards back to the full [b, n, D] output."""
    out = np.empty((b, n, D), dtype=np.float32)
    n_ii = n // 512
    for c in range(NCORES):
        oc = results[c]["out"]
        bi = c // 2
        for ii in range(n_ii):
            i0 = 512 * ii + 256 * (c % 2)
            out[bi, i0 : i0 + 256, :] = oc[ii * 256 : (ii + 1) * 256, :]
    return out


_NC_CACHE: dict = {}


def _get_nc(n=N, b=B):
    key = (n, b)
    if key not in _NC_CACHE:
        _NC_CACHE[key] = build_attention_bass(n, b)
    return _NC_CACHE[key]


def kernel(x, pos_bias, w_qkv, w_out, g_norm, g_out, _trace=False):
    x = np.asarray(x, dtype=np.float32)
    pos_bias = np.asarray(pos_bias, dtype=np.float32)
    w_qkv = np.asarray(w_qkv, dtype=np.float32)
    w_out = np.asarray(w_out, dtype=np.float32)
    g_norm = np.asarray(g_norm, dtype=np.float32)
    g_out = np.asarray(g_out, dtype=np.float32)
    b, n, _ = x.shape

    nc = _get_nc(n, b)
    in_maps = make_in_maps(x, pos_bias, w_qkv, w_out, g_norm, g_out, n, b)
    res = run_bass_kernel_spmd(
        nc, in_maps, core_ids=list(range(NCORES)), trace=_trace
    )
    if _trace:
        kernel.last_results = res
    return assemble_output(res.results, n, b)


# revision 43
# speedup vs baseline: 1.1019x; 1.0242x over previous
"""Trainium2 Bass kernel for imagen-style self-attention with pos_bias.

Reference computation (fp32 jax):
    xn   = LN(x) * g_norm                      # gamma-only layernorm
    qkv  = xn @ w_qkv ; q,k,v per head (h=8, d=64) ; q *= d**-0.5
    sim  = q @ k^T + pos_bias[h]               # [b, h, n, n]
    attn = softmax(sim, -1)
    out  = LN((attn @ v) @ w_out) * g_out

Distribution: 8 cores, one head per core (tensor parallel over heads).
Each core computes LN+QKV projection for its head over the full batch,
full attention for its head, then AllToAlls (split into n/512 chunks so
comm overlaps compute) re-shard by sequence rows; each core runs the
output projection for a 1/8 row shard; the final LN runs batched at the
end (keeps the scalar engine's Exp activation table resident during the
whole attention phase).

Row shard mapping: block (b, ii) = rows (b, i in [512*ii, 512*ii+512))
is split into two 256-row halves h; half (b, ii, h) goes to core 2b+h.
So core c owns rows (b=c//2, i in [512*ii + 256*(c%2), +256)) for all
ii, in ii order.

Softmax denominators travel with the AllToAll payload (row DH of each
[DH+1, 256] chunk); the attention output is scaled by 2^-6 pre-cast so
unnormalized f16 values cannot overflow, and normalization happens on
the receiving side (scale cancels in o/sums).
"""

import numpy as np

import concourse.bass as bass
import concourse.bacc as bacc
import concourse.mybir as mybir
import concourse.tile as tile
from concourse.bass_utils import run_bass_kernel_spmd
from concourse.masks import make_identity

B = 4
N = 2048
D = 512
HEADS = 8
DH = 64
SCALE = DH**-0.5
EPS = 1e-5
NCORES = 8
OSCALE = 2.0**-6

F16 = mybir.dt.float16
F32 = mybir.dt.float32
AF = mybir.ActivationFunctionType
ALU = mybir.AluOpType


def _phase1(nc, tc, x_d, w_sb, eps_t, ident, qT_sb, kT_sb, v_sb, n, b):
    """LN + transpose + QKV projection over all rows."""
    rows = b * n
    n_spans = rows // 512
    with (
        tc.tile_pool(name="p1", bufs=3) as p1,
        tc.tile_pool(name="p1xT", bufs=2) as p1xT,
        tc.tile_pool(name="ps_t", bufs=2, space="PSUM") as ps_t,
        tc.tile_pool(name="ps_p", bufs=2, space="PSUM") as ps_p,
    ):
        for sp in range(n_spans):
            xnT = p1xT.tile([128, 4, 512], F16, tag="xnT")
            mvs = p1.tile([128, 4, 2], F32, tag="mvs")
            rstds = p1.tile([128, 4], F32, tag="rstds")
            x_sp = p1.tile([128, 4, D], F16, tag="xsp")
            nc.sync.dma_start(
                out=x_sp,
                in_=x_d.rearrange("(q p) d -> p q d", p=128)[
                    :, sp * 4 : sp * 4 + 4, :
                ],
            )
            for t in range(4):
                stats = p1.tile([128, 6], F32, tag="stats")
                nc.vector.bn_stats(out=stats, in_=x_sp[:, t, :])
                nc.vector.bn_aggr(out=mvs[:, t, :], in_=stats)
            # batched rstd for the whole span: 1/sqrt(var + eps)
            nc.scalar.activation(
                out=rstds, in_=mvs[:, :, 1], func=AF.Sqrt, bias=eps_t
            )
            nc.vector.reciprocal(out=rstds, in_=rstds)
            for t in range(4):
                xn_t = p1.tile([128, D], F16, tag="xn")
                nc.vector.tensor_scalar(
                    out=xn_t,
                    in0=x_sp[:, t, :],
                    scalar1=mvs[:, t, 0:1],
                    scalar2=rstds[:, t : t + 1],
                    op0=ALU.subtract,
                    op1=ALU.mult,
                )
                # transpose 4 chunks into one psum bank, single evac cast
                ps = ps_t.tile([128, 512], F32, tag="tp")
                for c in range(4):
                    nc.tensor.matmul(
                        ps[:, c * 128 : (c + 1) * 128],
                        lhsT=xn_t[:, c * 128 : (c + 1) * 128],
                        rhs=ident,
                        start=(c == 0),
                        stop=(c == 3),
                    )
                nc.scalar.copy(
                    out=xnT[:, :, t * 128 : (t + 1) * 128],
                    in_=ps.rearrange("p (c w) -> p c w", c=4),
                )

            bi = (sp * 512) // n
            cols = slice((sp * 512) % n, (sp * 512) % n + 512)
            ps_q = ps_p.tile([64, 512], F32, tag="q")
            ps_k = ps_p.tile([64, 512], F32, tag="k")
            ps_v = ps_p.tile([64, 512], F32, tag="v")
            for c in range(4):
                st, fin = (c == 0), (c == 3)
                nc.tensor.matmul(
                    ps_q, lhsT=w_sb[:, c, 0:64], rhs=xnT[:, c, :], start=st, stop=fin
                )
                nc.tensor.matmul(
                    ps_k, lhsT=w_sb[:, c, 64:128], rhs=xnT[:, c, :], start=st, stop=fin
                )
                nc.tensor.matmul(
                    ps_v, lhsT=w_sb[:, c, 128:192], rhs=xnT[:, c, :], start=st, stop=fin
                )
            nc.vector.tensor_copy(out=qT_sb[:, bi, cols], in_=ps_q)
            nc.scalar.copy(out=kT_sb[:, bi, cols], in_=ps_k)
            vT_t = p1.tile([64, 512], F16, tag="vT")
            nc.scalar.copy(out=vT_t, in_=ps_v)
            # transpose vT [64, 512] into v natural [512, 64]: 4 transposes
            # into one psum bank, single strided evac
            jc0 = ((sp * 512) % n) // 128
            ps2 = ps_t.tile([128, 4, 64], F32, tag="tp", name="psv")
            for t in range(4):
                nc.tensor.matmul(
                    ps2[:, t, :],
                    lhsT=vT_t[:, t * 128 : (t + 1) * 128],
                    rhs=ident[0:64, 0:64],
                    start=(t == 0),
                    stop=(t == 3),
                )
            nc.scalar.copy(out=v_sb[:, bi, jc0 : jc0 + 4, 0:DH], in_=ps2)


def _load_pos(nc, p2c, post_d, n, ii, js=None):
    """Start DMA of the pos_bias column block for one i-span."""
    n_jc = n // 128
    pos_c = p2c.tile([128, n_jc, 512], F16, tag="posc")
    icols = slice(ii * 512, (ii + 1) * 512)
    srcv = post_d.rearrange("(j p) m -> p j m", p=128)
    if js is None:
        js = range(n_jc)
    nc.sync.dma_start(
        out=pos_c[:, js.start : js.stop, :],
        in_=srcv[:, js.start : js.stop, icols],
    )
    return pos_c


def _load_pos_rest(nc, pos_c, post_d, n, ii, js):
    icols = slice(ii * 512, (ii + 1) * 512)
    srcv = post_d.rearrange("(j p) m -> p j m", p=128)
    nc.sync.dma_start(
        out=pos_c[:, js.start : js.stop, :],
        in_=srcv[:, js.start : js.stop, icols],
    )


def _phase2(nc, pools, ident, qT_sb, kT_sb, v_sb, a2a_ins, n, b, ii, pos_c):
    """Attention for one i-span: S^T = pos^T + kT.T@qT ; exp ; O^T.

    The pos broadcast rides the PE (identity matmul into the S psum)
    deliberately: it keeps the PE the saturated rate-limiting engine,
    which holds the HAM clock gate open — scalar-bound variants let the
    PE idle per-pair and it gets throttled to 1.2 GHz, costing far more
    than the broadcast matmuls. j-chunk pairs share one [128, 2, 512] S
    psum (2 banks) so exp runs 1024 wide; O is left unnormalized
    (scaled by OSCALE) and the softmax sums row travels with the a2a
    payload."""
    n_jc = n // 128
    p2, p2o, ps_s, ps_o = pools
    icols = slice(ii * 512, (ii + 1) * 512)
    for bi in range(b):
        ps_O = ps_o.tile([DH + 1, 512], F32, tag="O")
        p_ts = {}

        def _o_mm(g):
            p_t = p_ts.pop(g)
            for h in range(2):
                j = 2 * g + h
                nc.tensor.matmul(
                    ps_O,
                    lhsT=v_sb[:, bi, j, :],
                    rhs=p_t[:, h, :],
                    start=(j == 0),
                    stop=(j == n_jc - 1),
                )

        for g in range(n_jc // 2):
            ps_S = ps_s.tile([128, 2, 512], F32, tag="S")
            p_t = p2.tile([128, 2, 512], F16, tag="P")
            p_ts[g] = p_t
            for h in range(2):
                nc.tensor.matmul(
                    ps_S[:, h, :],
                    lhsT=ident,
                    rhs=pos_c[:, 2 * g + h, :],
                    start=True,
                    stop=False,
                )
            for h in range(2):
                j = 2 * g + h
                nc.tensor.matmul(
                    ps_S[:, h, :],
                    lhsT=kT_sb[:, bi, j * 128 : (j + 1) * 128],
                    rhs=qT_sb[:, bi, icols],
                    start=False,
                    stop=True,
                )
            nc.scalar.activation(out=p_t, in_=ps_S, func=AF.Exp)
            if g >= 1:
                _o_mm(g - 1)
        _o_mm(n_jc // 2 - 1)
        o_t = p2o.tile([DH + 1, 512], F16, tag="onorm")
        nc.vector.tensor_scalar_mul(o_t, ps_O, OSCALE)
        for h in range(2):
            nc.sync.dma_start(
                out=a2a_ins[ii][2 * bi + h, :, :],
                in_=o_t[:, h * 256 : (h + 1) * 256],
            )


def _phase4_proj(nc, pools, src, wout_sb, sel_sb, y_sb, ci):
    """Out projection for one 256-row a2a chunk (normalize by sums row)."""
    p4, p4h, ps_r, ps_y = pools
    hT = p4h.tile([128, 4, 256], F16, tag="hT")
    srcv = src.rearrange("(c two) d r -> two d c r", two=2)
    for two in range(2):
        nc.sync.dma_start(
            out=hT[two * DH : (two + 1) * DH, :, :],
            in_=srcv[two, 0:DH, :, :],
        )
    sums = p4.tile([8, 256], F16, tag="sums")
    nc.sync.dma_start(out=sums, in_=src[:, DH, :])
    rec = p4.tile([8, 256], F16, tag="rec")
    with nc.allow_low_precision(reason="f16 softmax denom recip, tol 2e-2"):
        nc.vector.reciprocal(out=rec, in_=sums)
    hTn = p4h.tile([128, 4, 256], F16, tag="hTn")
    for cc in range(2):
        ps_rb = ps_r.tile([128, 2, 256], F32, tag="rb")
        for k in range(2):
            c = 2 * cc + k
            # broadcast rec rows into the partition layout of hT via PE:
            # sel_sb[:, c, :][s, p] = (s == 2c + p//64)
            nc.tensor.matmul(
                ps_rb[:, k, :], lhsT=sel_sb[:, c, :], rhs=rec, start=True, stop=True
            )
        for k in range(2):
            c = 2 * cc + k
            nc.vector.tensor_tensor(
                out=hTn[:, c, :], in0=hT[:, c, :], in1=ps_rb[:, k, :], op=ALU.mult
            )
    for it in range(2):
        ps = ps_y.tile([128, D], F32, tag="y")
        for c in range(4):
            nc.tensor.matmul(
                ps,
                lhsT=hTn[:, c, it * 128 : (it + 1) * 128],
                rhs=wout_sb[:, c, :],
                start=(c == 0),
                stop=(c == 3),
            )
        nc.vector.tensor_copy(out=y_sb[:, 2 * ci + it, :], in_=ps)


def _phase5_ln(nc, p5, y_sb, g_bc, eps_t, out_d, chunks):
    """Final LN over the given row chunks, batched (Sqrt table stays
    clear of the phase2 Exp stream)."""
    nch = len(chunks)
    mvs = p5.tile([128, 8, 2], F32, tag="mv5")
    for i, t in enumerate(chunks):
        stats = p5.tile([128, 6], F32, tag="st5")
        nc.vector.bn_stats(out=stats, in_=y_sb[:, t, :])
        nc.vector.bn_aggr(out=mvs[:, i, :], in_=stats)
    rstds = p5.tile([128, 8], F32, tag="rstd5")
    nc.scalar.activation(
        out=rstds[:, 0:nch], in_=mvs[:, 0:nch, 1], func=AF.Sqrt, bias=eps_t
    )
    nc.vector.reciprocal(out=rstds[:, 0:nch], in_=rstds[:, 0:nch])
    for i, t in enumerate(chunks):
        y_t = p5.tile([128, D], F32, tag="y5")
        nc.vector.tensor_scalar(
            out=y_t,
            in0=y_sb[:, t, :],
            scalar1=mvs[:, i, 0:1],
            scalar2=rstds[:, i : i + 1],
            op0=ALU.subtract,
            op1=ALU.mult,
        )
        nc.vector.tensor_tensor(out=y_t, in0=y_t, in1=g_bc, op=ALU.mult)
        nc.sync.dma_start(out=out_d[t * 128 : (t + 1) * 128, :], in_=y_t)


def build_attention_bass(n: int = N, b: int = B) -> bass.Bass:
    """Build the SPMD per-core Bass program (identical on all cores)."""
    rows = b * n
    assert rows % (NCORES * 128) == 0 and n % 512 == 0 and b == 4
    rows_pc = rows // NCORES
    n_ii = n // 512
    n_jc = n // 128

    nc = bacc.Bacc(num_devices=NCORES)

    x_d = nc.declare_dram_parameter("x", [rows, D], F16, isOutput=False)
    w_d = nc.declare_dram_parameter("w", [4, 128, 3 * DH], F16, isOutput=False)
    post_d = nc.declare_dram_parameter("post", [n, n], F16, isOutput=False)
    wout_d = nc.declare_dram_parameter("wout", [4, 128, D], F16, isOutput=False)
    g_d = nc.declare_dram_parameter("g", [1, D], F32, isOutput=False)
    sel_d = nc.declare_dram_parameter("sel", [4, 8, 128], F16, isOutput=False)
    out_d = nc.declare_dram_parameter("out", [rows_pc, D], F32, isOutput=True)

    a2a_ins = [
        nc.dram_tensor(f"a2a_in{ii}", [NCORES, DH + 1, 256], F16)
        for ii in range(n_ii)
    ]
    a2a_outs = [
        nc.dram_tensor(f"a2a_out{ii}", [NCORES, DH + 1, 256], F16)
        for ii in range(n_ii)
    ]
    warm_in = nc.dram_tensor("warm_in", [NCORES, 16], F16)
    warm_out = nc.dram_tensor("warm_out", [NCORES, 16], F16)

    groups = [list(range(NCORES))]

    with tile.TileContext(nc) as tc:
        with (
            tc.tile_pool(name="singles", bufs=1) as singles,
            tc.tile_pool(name="persist", bufs=1) as persist,
            tc.tile_pool(name="p2c", bufs=2) as p2c,
        ):
            # warm up the collective channels while phase1 computes
            nc.gpsimd.collective_compute(
                "AllToAll",
                ALU.bypass,
                replica_groups=groups,
                ins=[warm_in[:]],
                outs=[warm_out[:]],
            )
            ident = singles.tile([128, 128], F16)
            make_identity(nc, ident)
            eps_t = singles.tile([128, 1], F32)
            nc.vector.memset(eps_t, EPS)
            w_sb = singles.tile([128, 4, 3 * DH], F16)
            nc.sync.dma_start(out=w_sb, in_=w_d.rearrange("c p m -> p c m"))

            qT_sb = persist.tile([64, b, n], F16, name="qT")
            kT_sb = persist.tile([64, b, n], F16, name="kT")
            v_sb = persist.tile([128, b, n_jc, DH + 1], F16, name="v")
            nc.vector.memset(v_sb[:, :, :, DH : DH + 1], 1.0)

            # only the first j-chunks of pos[0] load before phase1 (keeps
            # the DMA queues clear for phase1's x tiles); the rest queue
            # right after phase1's emission, still well ahead of use
            pos_cur = _load_pos(nc, p2c, post_d, n, 0, js=range(0, 4))

            with (
                tc.tile_pool(name="p2", bufs=3) as p2,
                tc.tile_pool(name="p2o", bufs=2) as p2o,
                tc.tile_pool(name="p4", bufs=2) as p4,
                tc.tile_pool(name="p4s", bufs=1) as p4s,
                tc.tile_pool(name="p4h", bufs=2) as p4h,
                tc.tile_pool(name="p5", bufs=2) as p5,
            ):
                wout_sb = p4s.tile([128, 4, D], F16)
                nc.sync.dma_start(
                    out=wout_sb, in_=wout_d.rearrange("c p m -> p c m")
                )
                g_bc = p4s.tile([128, D], F32)
                nc.sync.dma_start(
                    out=g_bc, in_=g_d[0, :].partition_broadcast(128)
                )
                sel_sb = p4s.tile([8, 4, 128], F16)
                nc.sync.dma_start(
                    out=sel_sb, in_=sel_d.rearrange("c s p -> s c p")
                )
                y_sb = p4s.tile([128, 2 * n_ii, D], F32)
                P4LAG = 1
                _phase1(nc, tc, x_d, w_sb, eps_t, ident, qT_sb, kT_sb, v_sb,
                        n, b)
                _load_pos_rest(nc, pos_cur, post_d, n, 0, range(4, 16))
                with (
                    tc.tile_pool(name="ps_s", bufs=2, space="PSUM") as ps_s,
                    tc.tile_pool(name="ps_o", bufs=2, space="PSUM") as ps_o,
                    tc.tile_pool(name="ps_r", bufs=1, space="PSUM") as ps_r,
                    tc.tile_pool(name="ps_y", bufs=1, space="PSUM") as ps_y,
                ):
                    p2pools = (p2, p2o, ps_s, ps_o)
                    p4pools = (p4, p4h, ps_r, ps_y)
                    pos_tiles = {0: pos_cur, 1: _load_pos(
                        nc, p2c, post_d, n, 1)}
                    for ii in range(n_ii):
                        _phase2(
                            nc, p2pools, ident, qT_sb, kT_sb, v_sb, a2a_ins,
                            n, b, ii, pos_tiles.pop(ii),
                        )
                        nc.gpsimd.collective_compute(
                            "AllToAll",
                            ALU.bypass,
                            replica_groups=groups,
                            ins=[a2a_ins[ii][:]],
                            outs=[a2a_outs[ii][:]],
                        )
                        if ii >= P4LAG:
                            _phase4_proj(
                                nc, p4pools, a2a_outs[ii - P4LAG], wout_sb,
                                sel_sb, y_sb, ii - P4LAG,
                            )
                        if ii + 2 < n_ii:
                            # prefetch the after-next pos block here, after
                            # the collective trigger, so its DMA doesn't
                            # contend with the in-flight a2a transfer
                            pos_tiles[ii + 2] = _load_pos(
                                nc, p2c, post_d, n, ii + 2
                            )
                    # final LN for all but the last a2a chunk hides inside
                    # the last collective's transfer window
                    _phase5_ln(
                        nc, p5, y_sb, g_bc, eps_t, out_d,
                        list(range(2 * n_ii - 2)),
                    )
                    for ii in range(max(0, n_ii - P4LAG), n_ii):
                        _phase4_proj(
                            nc, p4pools, a2a_outs[ii], wout_sb, sel_sb,
                            y_sb, ii,
                        )
                    _phase5_ln(
                        nc, p5, y_sb, g_bc, eps_t, out_d,
                        [2 * n_ii - 2, 2 * n_ii - 1],
                    )

    nc.finalize()
    return nc


def make_in_maps(x, pos_bias, w_qkv, w_out, g_norm, g_out, n=N, b=B):
    """Host-side shard/layout prep: per-core input maps (no math beyond
    folding the LN gamma / attention scale diagonals into the weights)."""
    rows = b * n
    x16 = np.ascontiguousarray(x.reshape(rows, D)).astype(np.float16)
    w_eff = w_qkv * g_norm[:, None].astype(np.float32)
    wout16 = np.ascontiguousarray(w_out.reshape(4, 128, D)).astype(np.float16)
    g_row = np.ascontiguousarray(g_out.reshape(1, D)).astype(np.float32)
    sel = np.zeros((4, 8, 128), dtype=np.float16)
    for c in range(4):
        sel[c, 2 * c, 0:64] = 1.0
        sel[c, 2 * c + 1, 64:128] = 1.0
    hidden = HEADS * DH
    in_maps = []
    for h in range(NCORES):
        wq = w_eff[:, h * DH : (h + 1) * DH] * SCALE
        wk = w_eff[:, hidden + h * DH : hidden + (h + 1) * DH]
        wv = w_eff[:, 2 * hidden + h * DH : 2 * hidden + (h + 1) * DH]
        w_h = np.concatenate([wq, wk, wv], axis=1).reshape(4, 128, 3 * DH)
        posT = np.ascontiguousarray(pos_bias[h].T).astype(np.float16)
        in_maps.append(
            {
                "x": x16,
                "w": np.ascontiguousarray(w_h).astype(np.float16),
                "post": posT,
                "wout": wout16,
                "g": g_row,
                "sel": sel,
            }
        )
    return in_maps


def assemble_output(results, n=N, b=B):
    """Scatter per-core row shards back to the full [b, n, D] output."""
    out = np.empty((b, n, D), dtype=np.float32)
    n_ii = n // 512
    for c in range(NCORES):
        oc = results[c]["out"]
        bi = c // 2
        for ii in range(n_ii):
            i0 = 512 * ii + 256 * (c % 2)
            out[bi, i0 : i0 + 256, :] = oc[ii * 256 : (ii + 1) * 256, :]
    return out


_NC_CACHE: dict = {}


def _get_nc(n=N, b=B):
    key = (n, b)
    if key not in _NC_CACHE:
        _NC_CACHE[key] = build_attention_bass(n, b)
    return _NC_CACHE[key]


def kernel(x, pos_bias, w_qkv, w_out, g_norm, g_out, _trace=False):
    x = np.asarray(x, dtype=np.float32)
    pos_bias = np.asarray(pos_bias, dtype=np.float32)
    w_qkv = np.asarray(w_qkv, dtype=np.float32)
    w_out = np.asarray(w_out, dtype=np.float32)
    g_norm = np.asarray(g_norm, dtype=np.float32)
    g_out = np.asarray(g_out, dtype=np.float32)
    b, n, _ = x.shape

    nc = _get_nc(n, b)
    in_maps = make_in_maps(x, pos_bias, w_qkv, w_out, g_norm, g_out, n, b)
    res = run_bass_kernel_spmd(
        nc, in_maps, core_ids=list(range(NCORES)), trace=_trace
    )
    if _trace:
        kernel.last_results = res
    return assemble_output(res.results, n, b)
